# revision 15
# baseline (speedup 1.0000x reference)
"""Trainium2 Bass kernel for nn_DetectionHead (VoteNet-style detection head).

Self-contained: builds an 8-core SPMD Bass/Tile kernel, shards the M=128
clusters across cores (interleaved mod 8), replicates FPS + NMS, and
AllGathers the per-core box logits for the final NMS pass.

kernel(**inputs) takes the full unsharded inputs and returns the full
[128, 6] output.
"""

import numpy as np

NCORES = 8
N = 4096          # points
C = 128           # feature channels
M = 128           # clusters
MC = M // NCORES  # clusters per core (16)
NJ = 32           # FPS free-dim (N = 128 * NJ)
RADIUS = 0.5
THR = RADIUS * RADIUS   # 0.25 (d2 < THR)
NMS_THR = 0.25
BIG = 1.0e7
NMS_ITERS = 24
CHUNK = 512
NCHUNK = N // CHUNK       # 8
GRP = 4                   # psum2 groups of 4 chunks -> [128, 2048]

_cache = {}


def _build(debug=False):
    import concourse.bacc as bacc
    import concourse.tile as tile
    import concourse.mybir as mybir
    import concourse.bass_isa as bass_isa

    F32 = mybir.dt.float32
    F32R = mybir.dt.float32r
    I32 = mybir.dt.int32
    ALU = mybir.AluOpType
    ACTF = mybir.ActivationFunctionType
    AX = mybir.AxisListType

    nc = bacc.Bacc("TRN2", target_bir_lowering=False, debug=False,
                   num_devices=NCORES)

    # ---- DRAM I/O ----
    d_pts96 = nc.dram_tensor("pts96", [128, 96], F32, kind="ExternalInput")
    d_pT = nc.dram_tensor("pT", [3, N], F32, kind="ExternalInput")
    d_featT = nc.dram_tensor("featT", [C, N], F32, kind="ExternalInput")
    d_W1a = nc.dram_tensor("W1a", [3, C], F32, kind="ExternalInput")
    d_W1b = nc.dram_tensor("W1b", [C, C], F32, kind="ExternalInput")
    d_W2 = nc.dram_tensor("W2", [C, C], F32, kind="ExternalInput")
    d_W3 = nc.dram_tensor("W3", [C, C], F32, kind="ExternalInput")
    d_W4 = nc.dram_tensor("W4", [C, C], F32, kind="ExternalInput")
    d_Wf = nc.dram_tensor("Wf", [C, 7], F32, kind="ExternalInput")
    d_b1r = nc.dram_tensor("b1r", [1, C], F32, kind="ExternalInput")
    d_b2c = nc.dram_tensor("b2c", [C, 1], F32, kind="ExternalInput")
    d_b3c = nc.dram_tensor("b3c", [C, 1], F32, kind="ExternalInput")
    d_b4c = nc.dram_tensor("b4c", [C, 1], F32, kind="ExternalInput")
    d_bfr = nc.dram_tensor("bfr", [1, 7], F32, kind="ExternalInput")
    d_sel16 = nc.dram_tensor("sel16", [128, MC], F32, kind="ExternalInput")

    d_out = nc.dram_tensor("out", [M, 6], F32, kind="ExternalOutput")
    if debug:
        d_dbg_centers = nc.dram_tensor("dbg_centers", [M, 3], F32,
                                       kind="ExternalOutput")
        d_dbg_G = nc.dram_tensor("dbg_G", [C, MC], F32, kind="ExternalOutput")
        d_dbg_boxesT = nc.dram_tensor("dbg_boxesT", [7, MC], F32,
                                      kind="ExternalOutput")
        d_dbg_mask = nc.dram_tensor("dbg_mask", [MC, N], F32,
                                    kind="ExternalOutput")
        d_dbg_keep = nc.dram_tensor("dbg_keep", [M, 1], F32,
                                    kind="ExternalOutput")

    from contextlib import ExitStack
    es = ExitStack()
    with tile.TileContext(nc) as tc:
        cp = es.enter_context(tc.tile_pool(name="const", bufs=1))
        stage_es = ExitStack()
        stage_pool = stage_es.enter_context(tc.tile_pool(name="stage", bufs=1))
        featT_stage = stage_pool.tile([C, N], F32)
        # ---- constant / persistent tiles ----
        pts96 = cp.tile([128, 96], F32)
        pT = cp.tile([3, N], F32)
        pTsq = cp.tile([3, N], F32)
        featT = cp.tile([C, N], F32R)
        pTr = cp.tile([3, N], F32R)
        P3 = cp.tile([C, N], F32R)
        mask16 = cp.tile([MC, N], F32R)
        W1a = cp.tile([3, C], F32)
        W1am2 = cp.tile([3, C], F32R)
        W1br = cp.tile([C, C], F32R)
        W2r = cp.tile([C, C], F32R)
        W3r = cp.tile([C, C], F32R)
        W4r = cp.tile([C, C], F32R)
        Wfr = cp.tile([C, 7], F32R)
        W1b = cp.tile([C, C], F32)
        W2 = cp.tile([C, C], F32)
        W3 = cp.tile([C, C], F32)
        W4 = cp.tile([C, C], F32)
        Wf = cp.tile([C, 7], F32)
        b1r = cp.tile([1, C], F32)
        b2c = cp.tile([C, 1], F32)
        b3c = cp.tile([C, 1], F32)
        b4c = cp.tile([C, 1], F32)
        bfr = cp.tile([1, 7], F32)
        sel16 = cp.tile([128, MC], F32)
        ident = cp.tile([128, 128], F32)
        ident_i = cp.tile([128, 128], I32)
        ones_1x128 = cp.tile([1, 128], F32)
        ones_1x16 = cp.tile([1, MC], F32)
        ones_3x16 = cp.tile([3, MC], F32)
        negbig = cp.tile([1, C], F32)
        NB16 = cp.tile([MC, MC * 128], F32R)
        NB16_i = cp.tile([MC, MC * 128], I32)
        ER = cp.tile([8, 8 * 128], F32)
        ER_i = cp.tile([8, 8 * 128], I32)
        IDrow = cp.tile([1, 256], F32)
        IDrow_i = cp.tile([1, 256], I32)
        centers = cp.tile([128, 3], F32)
        U2b = cp.tile([C, MC], F32)
        ctm = cp.tile([3, MC], F32)    # centersT_mine
        ctm2 = cp.tile([3, MC], F32)   # -2 * centersT_mine
        cmine = cp.tile([MC, 3], F32)
        cmsq = cp.tile([MC, 3], F32)
        c2m = cp.tile([MC, 1], F32)
        negthr = cp.tile([MC, 1], F32)
        G = cp.tile([C, MC], F32)
        Grelu = cp.tile([C, MC], F32R)
        boxesT = cp.tile([7, MC], F32)
        # FPS state
        min_d = cp.tile([128, NJ], F32)
        d_newt = cp.tile([128, NJ], F32)
        rowmax = cp.tile([128, 1], F32)
        gb = cp.tile([128, 1], F32)
        partials = cp.tile([128, 3], F32)
        selbb = cp.tile([128, 3], F32)
        masked96 = cp.tile([128, 96], F32)
        diff96 = cp.tile([128, 96], F32)
        diffsq = cp.tile([128, 96], F32)
        iotaN_i = cp.tile([128, NJ], I32)
        # NMS tiles
        S14 = cp.tile([14, 128], F32)
        BX = cp.tile([128, 14], F32)
        PR = cp.tile([128, 8], F32)
        TPs = cp.tile([8, 128], F32)
        P_s = cp.tile([128, 128], F32)
        keep = cp.tile([128, 1], F32)
        lo3 = cp.tile([128, 3], F32)
        hi3 = cp.tile([128, 3], F32)
        vol = cp.tile([128, 1], F32)
        outt = cp.tile([128, 6], F32)

        # ---- input DMA ----
        nc.sync.dma_start(pts96[:], d_pts96.ap())
        nc.sync.dma_start(pT[:], d_pT.ap())
        nc.sync.dma_start(featT_stage[:], d_featT.ap())
        nc.sync.dma_start(W1a[:], d_W1a.ap())
        nc.sync.dma_start(W1b[:], d_W1b.ap())
        nc.sync.dma_start(W2[:], d_W2.ap())
        nc.sync.dma_start(W3[:], d_W3.ap())
        nc.sync.dma_start(W4[:], d_W4.ap())
        nc.sync.dma_start(Wf[:], d_Wf.ap())
        nc.sync.dma_start(b1r[:], d_b1r.ap())
        nc.sync.dma_start(b2c[:], d_b2c.ap())
        nc.sync.dma_start(b3c[:], d_b3c.ap())
        nc.sync.dma_start(b4c[:], d_b4c.ap())
        nc.sync.dma_start(bfr[:], d_bfr.ap())
        nc.sync.dma_start(sel16[:], d_sel16.ap())

        # ---- constants ----
        nc.gpsimd.iota(ident_i[:], pattern=[[1, 128]], base=0,
                       channel_multiplier=-1)
        nc.vector.tensor_scalar(ident[:], ident_i[:], 0, None,
                                op0=ALU.is_equal)
        nc.vector.memset(ones_1x128[:], 1.0)
        nc.vector.memset(ones_1x16[:], 1.0)
        nc.vector.memset(ones_3x16[:], 1.0)
        nc.vector.memset(negbig[:], -BIG)
        nc.gpsimd.iota(NB16_i[:].rearrange("p (j c) -> p j c", c=128),
                       pattern=[[1, MC], [0, 128]], base=0,
                       channel_multiplier=-1)
        nc.vector.tensor_scalar(NB16[:], NB16_i[:], 0, -BIG,
                                op0=ALU.is_equal, op1=ALU.mult)
        nc.gpsimd.iota(ER_i[:].rearrange("p (j c) -> p j c", c=128),
                       pattern=[[1, 8], [0, 128]], base=0,
                       channel_multiplier=-1)
        nc.vector.tensor_scalar(ER[:], ER_i[:], 0, None, op0=ALU.is_equal)
        nc.gpsimd.iota(IDrow_i[:], pattern=[[1, 256]], base=0,
                       channel_multiplier=0)
        nc.vector.tensor_scalar(IDrow[:], IDrow_i[:], 127, None,
                                op0=ALU.is_equal)
        nc.vector.tensor_scalar_mul(W1am2[:], W1a[:], -2.0)
        nc.vector.tensor_copy(featT[:], featT_stage[:])
        nc.vector.tensor_copy(W1br[:], W1b[:])
        nc.vector.tensor_copy(W2r[:], W2[:])
        nc.vector.tensor_copy(W3r[:], W3[:])
        nc.vector.tensor_copy(W4r[:], W4[:])
        nc.vector.tensor_copy(Wfr[:], Wf[:])
        nc.scalar.copy(pTr[:], pT[:])
        nc.gpsimd.iota(iotaN_i[:], pattern=[[1, NJ]], base=0,
                       channel_multiplier=NJ)
        nc.vector.tensor_scalar(min_d[:], iotaN_i[:], -1.0, None,
                                op0=ALU.mult)
        nc.vector.tensor_mul(pTsq[:], pT[:], pT[:])

        # views of pts96
        pts_pjc = pts96[:].rearrange("p (j c) -> p j c", c=3)

        stage_es.close()
        # ================= FPS =================
        fps_es = ExitStack()
        fps_psum = fps_es.enter_context(
            tc.tile_pool(name="fps_psum", bufs=2, space="PSUM"))
        p3_psum = fps_es.enter_context(
            tc.tile_pool(name="p3_psum", bufs=1, space="PSUM"))
        ctr_psum = fps_es.enter_context(
            tc.tile_pool(name="ctr_psum", bufs=1, space="PSUM"))
        srow_pool = fps_es.enter_context(tc.tile_pool(name="srow", bufs=3))
        centers_ps = ctr_psum.tile([128, 3], F32)

        # ---- P3 = W1b^T @ featT + (-2 W1a)^T @ pT  (cluster independent) ----
        for ci in range(NCHUNK):
            sl = slice(ci * CHUNK, (ci + 1) * CHUNK)
            ps = p3_psum.tile([C, CHUNK], F32, tag="p3ps")
            nc.tensor.matmul(ps[:], W1br[:], featT[:, sl], start=True,
                             stop=False)
            nc.tensor.matmul(ps[:], W1am2[:], pTr[:, sl], start=False,
                             stop=True)
            nc.scalar.copy(P3[:, sl], ps[:])

        def fps_select_update(t):
            """Select center t from min_d (+pending d_newt), update state."""
            if t == 0:
                pass  # min_d holds -n (selects point 0)
            elif t == 1:
                nc.vector.tensor_copy(min_d[:], d_newt[:])
            else:
                nc.vector.tensor_tensor(min_d[:], min_d[:], d_newt[:],
                                        op=ALU.min)
            nc.vector.tensor_reduce(rowmax[:], min_d[:], axis=AX.X,
                                    op=ALU.max)
            nc.gpsimd.partition_all_reduce(gb[:], rowmax[:], channels=128,
                                           reduce_op=bass_isa.ReduceOp.max)
            # masked96 = (min_d >= gmax) * pts   (global one-hot mask)
            nc.vector.scalar_tensor_tensor(
                out=masked96[:].rearrange("p (j c) -> p c j", c=3),
                in0=min_d[:].unsqueeze(1).broadcast_to([128, 3, NJ]),
                scalar=gb[:],
                in1=pts96[:].rearrange("p (j c) -> p c j", c=3),
                op0=ALU.is_ge, op1=ALU.mult)
            nc.vector.tensor_reduce(
                partials[:], masked96[:].rearrange("p (j c) -> p c j", c=3),
                axis=AX.X, op=ALU.add)
            nc.gpsimd.partition_all_reduce(selbb[:], partials[:], channels=128,
                                           reduce_op=bass_isa.ReduceOp.add)
            # record center: centers_ps += e_t (x) c_t
            srow = srow_pool.tile([1, 3], F32, tag="srow")
            nc.scalar.copy(srow[:], selbb[0:1, :])
            nc.tensor.matmul(centers_ps[:],
                             IDrow[0:1, 127 - t:255 - t], srow[:],
                             start=(t == 0), stop=(t == M - 1),
                             skip_group_check=True)
            if t == M - 1:
                return
            # d_newt = sum_c (pts - c_t)^2
            nc.vector.tensor_tensor(
                diff96[:].rearrange("p (j c) -> p j c", c=3),
                pts96[:].rearrange("p (j c) -> p j c", c=3),
                selbb[:].unsqueeze(1).broadcast_to([128, NJ, 3]),
                op=ALU.subtract)
            nc.vector.tensor_mul(diffsq[:], diff96[:], diff96[:])
            nc.vector.tensor_reduce(
                d_newt[:], diffsq[:].rearrange("p (j c) -> p j c", c=3),
                axis=AX.X, op=ALU.add)

        for t in range(M):
            fps_select_update(t)

        nc.scalar.copy(centers[:], centers_ps[:])
        fps_es.close()

        if debug:
            nc.sync.dma_start(d_dbg_centers.ap(), centers[:])

        # ================= post-FPS per-core prep =================
        sc_es = ExitStack()
        sc_psum = sc_es.enter_context(
            tc.tile_pool(name="sc_psum", bufs=2, space="PSUM"))

        # centers_mine [16, 3]
        ps_cm = sc_psum.tile([MC, 3], F32, tag="cm")
        nc.tensor.matmul(ps_cm[:], sel16[:], centers[:], start=True, stop=True)
        nc.scalar.copy(cmine[:], ps_cm[:])
        # c2 and thresholds
        nc.vector.tensor_mul(cmsq[:], cmine[:], cmine[:])
        nc.vector.tensor_reduce(c2m[:], cmsq[:], axis=AX.X, op=ALU.add)
        # negthr = c2 - THR
        nc.vector.tensor_scalar(negthr[:], c2m[:], -THR, None, op0=ALU.add)
        # centersT_mine [3, 16]
        ps_ctm = sc_psum.tile([3, MC], F32, tag="ctm")
        nc.tensor.transpose(ps_ctm[:], cmine[:], ident[0:MC, 0:MC])
        nc.scalar.copy(ctm[:], ps_ctm[:])
        nc.scalar.mul(ctm2[:], ps_ctm[:], -2.0)
        # U2b [128, 16] = W1a^T @ centersT_mine + b1
        ps_u = sc_psum.tile([C, MC], F32, tag="u2b")
        nc.tensor.matmul(ps_u[:], W1a[:], ctm[:], start=True, stop=False)
        nc.tensor.matmul(ps_u[:], b1r[:], ones_1x16[:], start=False, stop=True)
        nc.scalar.copy(U2b[:], ps_u[:])
        # mask16 [16, N]: relu(q - 2 c.p + c2 - THR)  (0 iff valid)
        for ci in range(NCHUNK):
            sl = slice(ci * CHUNK, (ci + 1) * CHUNK)
            ps_m = sc_psum.tile([MC, CHUNK], F32, tag="m16")
            nc.tensor.matmul(ps_m[:], ones_3x16[:], pTsq[:, sl], start=True,
                             stop=False)
            nc.tensor.matmul(ps_m[:], ctm2[:], pT[:, sl], start=False,
                             stop=True)
            nc.scalar.activation(mask16[:, sl], ps_m[:], ACTF.Relu,
                                 bias=negthr[:], scale=1.0)
        sc_es.close()

        if debug:
            nc.sync.dma_start(d_dbg_mask.ap(), mask16[:])

        # ================= per-cluster MLP + masked max =================
        mlp_es = ExitStack()
        mlp_psum = mlp_es.enter_context(
            tc.tile_pool(name="mlp_psum", bufs=2, space="PSUM"))
        h1_pool = mlp_es.enter_context(tc.tile_pool(name="h1", bufs=6))
        gp_pool = mlp_es.enter_context(tc.tile_pool(name="gp", bufs=2))
        for j in range(MC):
            gparts = gp_pool.tile([C, 2], F32, tag="gparts")
            for grp in range(2):
                ps2 = mlp_psum.tile([C, GRP * CHUNK], F32, tag="ps2")
                h1list = []
                for q in range(GRP):
                    ci = grp * GRP + q
                    sl = slice(ci * CHUNK, (ci + 1) * CHUNK)
                    h1 = h1_pool.tile([C, CHUNK], F32R, tag="h1")
                    if q % 2 == 0:
                        nc.scalar.activation(h1[:], P3[:, sl], ACTF.Relu,
                                             bias=U2b[:, j:j + 1], scale=1.0)
                    else:
                        nc.gpsimd.tensor_scalar(h1[:], P3[:, sl],
                                                U2b[:, j:j + 1], 0.0,
                                                op0=ALU.add, op1=ALU.max)
                    h1list.append(h1)
                for q in range(GRP):
                    qsl = slice(q * CHUNK, (q + 1) * CHUNK)
                    nc.tensor.matmul(ps2[:, qsl], W2r[:], h1list[q][:],
                                     start=True, stop=False)
                for q in range(GRP):
                    ci = grp * GRP + q
                    sl = slice(ci * CHUNK, (ci + 1) * CHUNK)
                    qsl = slice(q * CHUNK, (q + 1) * CHUNK)
                    nc.tensor.matmul(ps2[:, qsl],
                                     NB16[:, j * 128:(j + 1) * 128],
                                     mask16[:, sl], start=False,
                                     stop=True)
                nc.vector.tensor_reduce(gparts[:, grp:grp + 1], ps2[:],
                                        axis=AX.X, op=ALU.max)
            nc.vector.tensor_reduce(G[:, j:j + 1], gparts[:], axis=AX.X,
                                    op=ALU.max)
        mlp_es.close()

        if debug:
            nc.sync.dma_start(d_dbg_G.ap(), G[:])

        # ================= box MLP =================
        bx_es = ExitStack()
        bx_psum = bx_es.enter_context(
            tc.tile_pool(name="bx_psum", bufs=1, space="PSUM"))
        bx_pool = bx_es.enter_context(tc.tile_pool(name="bx", bufs=2))
        # g = relu(gmax + b2)
        nc.scalar.activation(Grelu[:], G[:], ACTF.Relu, bias=b2c[:], scale=1.0)
        ps_g3 = bx_psum.tile([C, MC], F32, tag="g3")
        nc.tensor.matmul(ps_g3[:], W3r[:], Grelu[:], start=True, stop=True)
        g3 = bx_pool.tile([C, MC], F32R, tag="g3s")
        nc.scalar.activation(g3[:], ps_g3[:], ACTF.Relu, bias=b3c[:],
                             scale=1.0)
        ps_g4 = bx_psum.tile([C, MC], F32, tag="g4")
        nc.tensor.matmul(ps_g4[:], W4r[:], g3[:], start=True, stop=True)
        g4 = bx_pool.tile([C, MC], F32R, tag="g4s")
        nc.scalar.activation(g4[:], ps_g4[:], ACTF.Relu, bias=b4c[:],
                             scale=1.0)
        ps_bx = bx_psum.tile([7, MC], F32, tag="bx")
        nc.tensor.matmul(ps_bx[:], Wfr[:], g4[:], start=True, stop=False)
        nc.tensor.matmul(ps_bx[:], bfr[:], ones_1x16[:], start=False,
                         stop=True)
        nc.scalar.copy(boxesT[:], ps_bx[:])
        bx_es.close()

        if debug:
            nc.sync.dma_start(d_dbg_boxesT.ap(), boxesT[:])

        # ================= AllGather box logits =================
        dram = es.enter_context(tc.tile_pool(name="dram", bufs=1, space="DRAM"))
        bounce_in = dram.tile([7, MC], F32)
        bounce_out = dram.tile([NCORES, 7 * MC], F32)
        nc.sync.dma_start(bounce_in[:], boxesT[:])
        nc.gpsimd.collective_compute(
            "AllGather", mybir.AluOpType.bypass,
            replica_groups=[list(range(NCORES))],
            ins=[bounce_in[:].opt()],
            outs=[bounce_out[:].opt()],
        )
        # reassemble: cluster m = 8j+k -> bounce_out[k, c*16+j]
        BTall = cp.tile([7, 128], F32)
        nc.sync.dma_start(
            BTall[:].rearrange("c (j k) -> c j k", k=NCORES),
            bounce_out[:].rearrange("k (c j) -> c j k", j=MC),
        )

        # ================= NMS =================
        nms_es = ExitStack()
        nms_psum = nms_es.enter_context(
            tc.tile_pool(name="nms_psum", bufs=1, space="PSUM"))
        # S7 = sigmoid(logits); BX = [sig | logits] transposed
        nc.scalar.activation(S14[0:7, :], BTall[:], ACTF.Sigmoid)
        ps_bxall = nms_psum.tile([128, 14], F32, tag="bxall")
        nc.tensor.transpose(ps_bxall[:, 0:7], S14[0:7, :], ident[0:7, 0:7])
        nc.tensor.transpose(ps_bxall[:, 7:14], BTall[:], ident[0:7, 0:7])
        nc.vector.tensor_copy(BX[:], ps_bxall[:])
        # cols of BX: 0 score-sig, 1..3 center, 4..6 dims, 7 score-logit
        # lo = c - 0.5 d ; hi = c + 0.5 d
        nc.vector.scalar_tensor_tensor(lo3[:], BX[:, 4:7], -0.5, BX[:, 1:4],
                                       op0=ALU.mult, op1=ALU.add)
        nc.vector.scalar_tensor_tensor(hi3[:], BX[:, 4:7], 0.5, BX[:, 1:4],
                                       op0=ALU.mult, op1=ALU.add)
        nc.vector.tensor_mul(vol[:], BX[:, 4:5], BX[:, 5:6])
        nc.vector.tensor_mul(vol[:], vol[:], BX[:, 6:7])
        # PR = [lo3 | hi3 | vol | score-logit]
        nc.vector.tensor_copy(PR[:, 0:3], lo3[:])
        nc.vector.tensor_copy(PR[:, 3:6], hi3[:])
        nc.vector.tensor_copy(PR[:, 6:7], vol[:])
        nc.vector.tensor_copy(PR[:, 7:8], BX[:, 7:8])
        ps_tp = nms_psum.tile([8, 128], F32, tag="tp")
        nc.tensor.transpose(ps_tp[:], PR[:], ident[:])
        nc.vector.tensor_copy(TPs[:], ps_tp[:])
        # broadcast all 8 rows: psumB[:, r*128:(r+1)*128] = row r over parts
        psB = nms_psum.tile([128, 8 * 128], F32, tag="psB")
        for r in range(8):
            nc.tensor.matmul(psB[:, r * 128:(r + 1) * 128],
                             ER[:, r * 128:(r + 1) * 128],
                             TPs[:], start=True, stop=True)

        def colB(r):
            return psB[:, r * 128:(r + 1) * 128]

        wrk = nms_es.enter_context(tc.tile_pool(name="nms_wrk", bufs=1))
        inter = wrk.tile([128, 128], F32, tag="inter")
        tmpA = wrk.tile([128, 128], F32, tag="tmpA")
        tmpB = wrk.tile([128, 128], F32, tag="tmpB")
        for c in range(3):
            # min(hi_i, hi_j)
            nc.vector.tensor_scalar(tmpA[:], colB(3 + c), hi3[:, c:c + 1],
                                    None, op0=ALU.min)
            # max(lo_i, lo_j)
            nc.vector.tensor_scalar(tmpB[:], colB(c), lo3[:, c:c + 1], None,
                                    op0=ALU.max)
            # w = relu(minhi - maxlo)
            nc.vector.scalar_tensor_tensor(tmpA[:], tmpB[:], -1.0, tmpA[:],
                                           op0=ALU.mult, op1=ALU.add)
            nc.vector.tensor_scalar_max(tmpA[:], tmpA[:], 0.0)
            if c == 0:
                nc.vector.tensor_copy(inter[:], tmpA[:])
            else:
                nc.vector.tensor_mul(inter[:], inter[:], tmpA[:])
        # volsum = vol_i + vol_j + 1e-8
        nc.vector.tensor_scalar(tmpB[:], colB(6), vol[:], 1e-8, op0=ALU.add,
                                op1=ALU.add)
        # D = volsum - inter
        nc.vector.scalar_tensor_tensor(tmpB[:], inter[:], -1.0, tmpB[:],
                                       op0=ALU.mult, op1=ALU.add)
        # P_iou = (4*inter > D)
        nc.vector.scalar_tensor_tensor(tmpA[:], inter[:], 1.0 / NMS_THR,
                                       tmpB[:], op0=ALU.mult, op1=ALU.is_gt)
        # P_score[i,j] = score_j < score_i
        nc.vector.tensor_scalar(tmpB[:], colB(7), BX[:, 7:8], None,
                                op0=ALU.is_lt)
        nc.vector.tensor_mul(P_s[:], tmpA[:], tmpB[:])
        # Jacobi fixpoint: keep_j = !any_i P[i,j] keep_i
        nc.vector.memset(keep[:], 1.0)
        ps_k = nms_psum.tile([128, 1], F32, tag="kps")
        for it in range(NMS_ITERS):
            nc.tensor.matmul(ps_k[:], P_s[:], keep[:], start=True, stop=True)
            nc.vector.tensor_scalar(keep[:], ps_k[:], 0.5, None, op0=ALU.is_lt)
        if debug:
            nc.sync.dma_start(d_dbg_keep.ap(), keep[:])
        # out = coords * keep
        nc.vector.tensor_scalar(outt[:], BX[:, 1:7], keep[:], None,
                                op0=ALU.mult)
        nc.sync.dma_start(d_out.ap(), outt[:])

        nms_es.close()
        es.close()

    nc.compile()
    return nc


def _prep_inputs(vote_points, vote_features, W1, b1, W2, b2, W3, b3, W4, b4,
                 Wf, bf):
    """Pure layout transforms of the full inputs -> per-core input maps."""
    f32 = np.float32
    pts = np.ascontiguousarray(vote_points, dtype=f32)
    feat = np.ascontiguousarray(vote_features, dtype=f32)
    base = {
        "pts96": pts.reshape(128, 96).copy(),
        "pT": pts.T.copy(),
        "featT": feat.T.copy(),
        "W1a": np.ascontiguousarray(W1[:3], f32),
        "W1b": np.ascontiguousarray(W1[3:], f32),
        "W2": np.ascontiguousarray(W2, f32),
        "W3": np.ascontiguousarray(W3, f32),
        "W4": np.ascontiguousarray(W4, f32),
        "Wf": np.ascontiguousarray(Wf, f32),
        "b1r": np.ascontiguousarray(b1, f32).reshape(1, C),
        "b2c": np.ascontiguousarray(b2, f32).reshape(C, 1),
        "b3c": np.ascontiguousarray(b3, f32).reshape(C, 1),
        "b4c": np.ascontiguousarray(b4, f32).reshape(C, 1),
        "bfr": np.ascontiguousarray(bf, f32).reshape(1, 7),
    }
    in_maps = []
    for k in range(NCORES):
        m = dict(base)
        sel = np.zeros((128, MC), f32)
        for j in range(MC):
            sel[NCORES * j + k, j] = 1.0
        m["sel16"] = sel
        in_maps.append(m)
    return in_maps


def kernel(**inputs):
    from concourse.bass_utils import run_bass_kernel_spmd

    if "nc" not in _cache:
        _cache["nc"] = _build(debug=False)
    nc = _cache["nc"]
    in_maps = _prep_inputs(**inputs)
    res = run_bass_kernel_spmd(nc, in_maps, core_ids=list(range(NCORES)))
    out = np.asarray(res.results[0]["out"], dtype=np.float32)
    return out


# revision 16
# speedup vs baseline: 1.4411x; 1.4411x over previous
"""Trainium2 Bass kernel for nn_DetectionHead (VoteNet-style detection head).

Self-contained: builds an 8-core SPMD Bass/Tile kernel, shards the M=128
clusters across cores (interleaved mod 8), replicates FPS + NMS, and
AllGathers the per-core box logits for the final NMS pass.

kernel(**inputs) takes the full unsharded inputs and returns the full
[128, 6] output.
"""

import numpy as np

NCORES = 8
N = 4096          # points
C = 128           # feature channels
M = 128           # clusters
MC = M // NCORES  # clusters per core (16)
NJ = 32           # FPS free-dim (N = 128 * NJ)
RADIUS = 0.5
THR = RADIUS * RADIUS   # 0.25 (d2 < THR)
NMS_THR = 0.25
BIG = 1.0e7
NMS_ITERS = 16
CHUNK = 512
NCHUNK = N // CHUNK       # 8
GRP = 4                   # psum2 groups of 4 chunks -> [128, 2048]

_cache = {}


def _build(debug=False):
    import concourse.bacc as bacc
    import concourse.tile as tile
    import concourse.mybir as mybir
    import concourse.bass_isa as bass_isa

    F32 = mybir.dt.float32
    F32R = mybir.dt.float32r
    I32 = mybir.dt.int32
    ALU = mybir.AluOpType
    ACTF = mybir.ActivationFunctionType
    AX = mybir.AxisListType

    nc = bacc.Bacc("TRN2", target_bir_lowering=False, debug=False,
                   num_devices=NCORES)

    # ---- DRAM I/O ----
    d_pts96 = nc.dram_tensor("pts96", [128, 96], F32, kind="ExternalInput")
    d_pT = nc.dram_tensor("pT", [3, N], F32, kind="ExternalInput")
    d_featT = nc.dram_tensor("featT", [C, N], F32, kind="ExternalInput")
    d_W1a = nc.dram_tensor("W1a", [3, C], F32, kind="ExternalInput")
    d_W1b = nc.dram_tensor("W1b", [C, C], F32, kind="ExternalInput")
    d_W2 = nc.dram_tensor("W2", [C, C], F32, kind="ExternalInput")
    d_W3 = nc.dram_tensor("W3", [C, C], F32, kind="ExternalInput")
    d_W4 = nc.dram_tensor("W4", [C, C], F32, kind="ExternalInput")
    d_Wf = nc.dram_tensor("Wf", [C, 7], F32, kind="ExternalInput")
    d_b1r = nc.dram_tensor("b1r", [1, C], F32, kind="ExternalInput")
    d_b2c = nc.dram_tensor("b2c", [C, 1], F32, kind="ExternalInput")
    d_b3c = nc.dram_tensor("b3c", [C, 1], F32, kind="ExternalInput")
    d_b4c = nc.dram_tensor("b4c", [C, 1], F32, kind="ExternalInput")
    d_bfr = nc.dram_tensor("bfr", [1, 7], F32, kind="ExternalInput")
    d_sel16 = nc.dram_tensor("sel16", [128, MC], F32, kind="ExternalInput")

    d_out = nc.dram_tensor("out", [M, 6], F32, kind="ExternalOutput")
    if debug:
        d_dbg_centers = nc.dram_tensor("dbg_centers", [M, 3], F32,
                                       kind="ExternalOutput")
        d_dbg_G = nc.dram_tensor("dbg_G", [C, MC], F32, kind="ExternalOutput")
        d_dbg_boxesT = nc.dram_tensor("dbg_boxesT", [7, MC], F32,
                                      kind="ExternalOutput")
        d_dbg_mask = nc.dram_tensor("dbg_mask", [MC, N], F32,
                                    kind="ExternalOutput")
        d_dbg_keep = nc.dram_tensor("dbg_keep", [M, 1], F32,
                                    kind="ExternalOutput")

    from contextlib import ExitStack
    es = ExitStack()
    with tile.TileContext(nc) as tc:
        cp = es.enter_context(tc.tile_pool(name="const", bufs=1))
        stage_es = ExitStack()
        stage_pool = stage_es.enter_context(tc.tile_pool(name="stage", bufs=1))
        featT_stage = stage_pool.tile([C, N], F32)
        # ---- constant / persistent tiles ----
        pts96 = cp.tile([128, 96], F32)
        pT = cp.tile([3, N], F32)
        pTsq = cp.tile([3, N], F32)
        featT = cp.tile([C, N], F32R)
        pTr = cp.tile([3, N], F32R)
        P3 = cp.tile([C, N], F32R)
        mask16 = cp.tile([MC, N], F32R)
        W1a = cp.tile([3, C], F32)
        W1am2 = cp.tile([3, C], F32R)
        W1br = cp.tile([C, C], F32R)
        W2r = cp.tile([C, C], F32R)
        W3r = cp.tile([C, C], F32R)
        W4r = cp.tile([C, C], F32R)
        Wfr = cp.tile([C, 7], F32R)
        W1b = cp.tile([C, C], F32)
        W2 = cp.tile([C, C], F32)
        W3 = cp.tile([C, C], F32)
        W4 = cp.tile([C, C], F32)
        Wf = cp.tile([C, 7], F32)
        b1r = cp.tile([1, C], F32)
        b2c = cp.tile([C, 1], F32)
        b3c = cp.tile([C, 1], F32)
        b4c = cp.tile([C, 1], F32)
        bfr = cp.tile([1, 7], F32)
        sel16 = cp.tile([128, MC], F32)
        ident = cp.tile([128, 128], F32)
        ident_i = cp.tile([128, 128], I32)
        ones_1x128 = cp.tile([1, 128], F32)
        ones_1x16 = cp.tile([1, MC], F32)
        ones_3x16 = cp.tile([3, MC], F32)
        negbig = cp.tile([1, C], F32)
        NB16 = cp.tile([MC, MC * 128], F32R)
        NB16_i = cp.tile([MC, MC * 128], I32)
        ER = cp.tile([8, 8 * 128], F32)
        ER_i = cp.tile([8, 8 * 128], I32)
        IDrow = cp.tile([1, 256], F32)
        IDrow_i = cp.tile([1, 256], I32)
        centers = cp.tile([128, 3], F32)
        U2b = cp.tile([C, MC], F32)
        ctm = cp.tile([3, MC], F32)    # centersT_mine
        ctm2 = cp.tile([3, MC], F32)   # -2 * centersT_mine
        cmine = cp.tile([MC, 3], F32)
        cmsq = cp.tile([MC, 3], F32)
        c2m = cp.tile([MC, 1], F32)
        negthr = cp.tile([MC, 1], F32)
        G = cp.tile([C, MC], F32)
        Grelu = cp.tile([C, MC], F32R)
        boxesT = cp.tile([7, MC], F32)
        # FPS state
        min_d = cp.tile([128, NJ], F32)
        d_newt = cp.tile([128, NJ], F32)
        rowmax = cp.tile([128, 1], F32)
        gb = cp.tile([128, 1], F32)
        partials = cp.tile([128, 3], F32)
        selbb = cp.tile([128, 3], F32)
        masked96 = cp.tile([128, 96], F32)
        diff96 = cp.tile([128, 96], F32)
        diffsq = cp.tile([128, 96], F32)
        iotaN_i = cp.tile([128, NJ], I32)
        # NMS tiles
        S14 = cp.tile([14, 128], F32)
        BX = cp.tile([128, 14], F32)
        PR = cp.tile([128, 8], F32)
        TPs = cp.tile([8, 128], F32)
        P_s = cp.tile([128, 128], F32)
        keep = cp.tile([128, 1], F32)
        lo3 = cp.tile([128, 3], F32)
        hi3 = cp.tile([128, 3], F32)
        vol = cp.tile([128, 1], F32)
        outt = cp.tile([128, 6], F32)

        # ---- input DMA ----
        nc.sync.dma_start(pts96[:], d_pts96.ap())
        nc.sync.dma_start(pT[:], d_pT.ap())
        nc.sync.dma_start(featT_stage[:], d_featT.ap())
        nc.sync.dma_start(W1a[:], d_W1a.ap())
        nc.sync.dma_start(W1b[:], d_W1b.ap())
        nc.sync.dma_start(W2[:], d_W2.ap())
        nc.sync.dma_start(W3[:], d_W3.ap())
        nc.sync.dma_start(W4[:], d_W4.ap())
        nc.sync.dma_start(Wf[:], d_Wf.ap())
        nc.sync.dma_start(b1r[:], d_b1r.ap())
        nc.sync.dma_start(b2c[:], d_b2c.ap())
        nc.sync.dma_start(b3c[:], d_b3c.ap())
        nc.sync.dma_start(b4c[:], d_b4c.ap())
        nc.sync.dma_start(bfr[:], d_bfr.ap())
        nc.sync.dma_start(sel16[:], d_sel16.ap())

        # ---- constants ----
        nc.gpsimd.iota(ident_i[:], pattern=[[1, 128]], base=0,
                       channel_multiplier=-1)
        nc.vector.tensor_scalar(ident[:], ident_i[:], 0, None,
                                op0=ALU.is_equal)
        nc.vector.memset(ones_1x128[:], 1.0)
        nc.vector.memset(ones_1x16[:], 1.0)
        nc.vector.memset(ones_3x16[:], 1.0)
        nc.vector.memset(negbig[:], -BIG)
        nc.gpsimd.iota(NB16_i[:].rearrange("p (j c) -> p j c", c=128),
                       pattern=[[1, MC], [0, 128]], base=0,
                       channel_multiplier=-1)
        nc.vector.tensor_scalar(NB16[:], NB16_i[:], 0, -BIG,
                                op0=ALU.is_equal, op1=ALU.mult)
        nc.gpsimd.iota(ER_i[:].rearrange("p (j c) -> p j c", c=128),
                       pattern=[[1, 8], [0, 128]], base=0,
                       channel_multiplier=-1)
        nc.vector.tensor_scalar(ER[:], ER_i[:], 0, None, op0=ALU.is_equal)
        nc.gpsimd.iota(IDrow_i[:], pattern=[[1, 256]], base=0,
                       channel_multiplier=0)
        nc.vector.tensor_scalar(IDrow[:], IDrow_i[:], 127, None,
                                op0=ALU.is_equal)
        nc.vector.tensor_scalar_mul(W1am2[:], W1a[:], -2.0)
        nc.vector.tensor_copy(featT[:], featT_stage[:])
        nc.vector.tensor_copy(W1br[:], W1b[:])
        nc.vector.tensor_copy(W2r[:], W2[:])
        nc.vector.tensor_copy(W3r[:], W3[:])
        nc.vector.tensor_copy(W4r[:], W4[:])
        nc.vector.tensor_copy(Wfr[:], Wf[:])
        nc.scalar.copy(pTr[:], pT[:])
        nc.gpsimd.iota(iotaN_i[:], pattern=[[1, NJ]], base=0,
                       channel_multiplier=NJ)
        nc.vector.tensor_scalar(min_d[:], iotaN_i[:], -1.0, None,
                                op0=ALU.mult)
        nc.vector.tensor_mul(pTsq[:], pT[:], pT[:])

        # views of pts96
        pts_pjc = pts96[:].rearrange("p (j c) -> p j c", c=3)

        stage_es.close()
        # ================= FPS =================
        fps_es = ExitStack()
        fps_psum = fps_es.enter_context(
            tc.tile_pool(name="fps_psum", bufs=2, space="PSUM"))
        p3_psum = fps_es.enter_context(
            tc.tile_pool(name="p3_psum", bufs=1, space="PSUM"))
        ctr_psum = fps_es.enter_context(
            tc.tile_pool(name="ctr_psum", bufs=1, space="PSUM"))
        srow_pool = fps_es.enter_context(tc.tile_pool(name="srow", bufs=3))
        centers_ps = ctr_psum.tile([128, 3], F32)

        # ---- P3 = W1b^T @ featT + (-2 W1a)^T @ pT  (cluster independent) ----
        for ci in range(NCHUNK):
            sl = slice(ci * CHUNK, (ci + 1) * CHUNK)
            ps = p3_psum.tile([C, CHUNK], F32, tag="p3ps")
            nc.tensor.matmul(ps[:], W1br[:], featT[:, sl], start=True,
                             stop=False)
            nc.tensor.matmul(ps[:], W1am2[:], pTr[:, sl], start=False,
                             stop=True)
            nc.scalar.copy(P3[:, sl], ps[:])

        def fps_select_update(t):
            """Select center t from min_d (+pending d_newt), update state."""
            if t == 0:
                pass  # min_d holds -n (selects point 0)
            elif t == 1:
                nc.vector.tensor_copy(min_d[:], d_newt[:])
            else:
                nc.vector.tensor_tensor(min_d[:], min_d[:], d_newt[:],
                                        op=ALU.min)
            nc.vector.tensor_reduce(rowmax[:], min_d[:], axis=AX.X,
                                    op=ALU.max)
            nc.gpsimd.partition_all_reduce(gb[:], rowmax[:], channels=128,
                                           reduce_op=bass_isa.ReduceOp.max)
            # masked96 = (min_d >= gmax) * pts   (global one-hot mask)
            nc.vector.scalar_tensor_tensor(
                out=masked96[:].rearrange("p (j c) -> p c j", c=3),
                in0=min_d[:].unsqueeze(1).broadcast_to([128, 3, NJ]),
                scalar=gb[:],
                in1=pts96[:].rearrange("p (j c) -> p c j", c=3),
                op0=ALU.is_ge, op1=ALU.mult)
            nc.vector.tensor_reduce(
                partials[:], masked96[:].rearrange("p (j c) -> p c j", c=3),
                axis=AX.X, op=ALU.add)
            nc.gpsimd.partition_all_reduce(selbb[:], partials[:], channels=128,
                                           reduce_op=bass_isa.ReduceOp.add)
            # record center: centers_ps += e_t (x) c_t
            srow = srow_pool.tile([1, 3], F32, tag="srow")
            nc.scalar.copy(srow[:], selbb[0:1, :])
            nc.tensor.matmul(centers_ps[:],
                             IDrow[0:1, 127 - t:255 - t], srow[:],
                             start=(t == 0), stop=(t == M - 1),
                             skip_group_check=True)
            if t == M - 1:
                return
            # d_newt = sum_c (pts - c_t)^2
            nc.vector.tensor_tensor(
                diff96[:].rearrange("p (j c) -> p j c", c=3),
                pts96[:].rearrange("p (j c) -> p j c", c=3),
                selbb[:].unsqueeze(1).broadcast_to([128, NJ, 3]),
                op=ALU.subtract)
            nc.vector.tensor_mul(diffsq[:], diff96[:], diff96[:])
            nc.vector.tensor_reduce(
                d_newt[:], diffsq[:].rearrange("p (j c) -> p j c", c=3),
                axis=AX.X, op=ALU.add)

        for t in range(M):
            fps_select_update(t)

        nc.scalar.copy(centers[:], centers_ps[:])
        fps_es.close()

        if debug:
            nc.sync.dma_start(d_dbg_centers.ap(), centers[:])

        # ================= post-FPS per-core prep =================
        sc_es = ExitStack()
        sc_psum = sc_es.enter_context(
            tc.tile_pool(name="sc_psum", bufs=2, space="PSUM"))

        # centers_mine [16, 3]
        ps_cm = sc_psum.tile([MC, 3], F32, tag="cm")
        nc.tensor.matmul(ps_cm[:], sel16[:], centers[:], start=True, stop=True)
        nc.scalar.copy(cmine[:], ps_cm[:])
        # c2 and thresholds
        nc.vector.tensor_mul(cmsq[:], cmine[:], cmine[:])
        nc.vector.tensor_reduce(c2m[:], cmsq[:], axis=AX.X, op=ALU.add)
        # negthr = c2 - THR
        nc.vector.tensor_scalar(negthr[:], c2m[:], -THR, None, op0=ALU.add)
        # centersT_mine [3, 16]
        ps_ctm = sc_psum.tile([3, MC], F32, tag="ctm")
        nc.tensor.transpose(ps_ctm[:], cmine[:], ident[0:MC, 0:MC])
        nc.scalar.copy(ctm[:], ps_ctm[:])
        nc.scalar.mul(ctm2[:], ps_ctm[:], -2.0)
        # U2b [128, 16] = W1a^T @ centersT_mine + b1
        ps_u = sc_psum.tile([C, MC], F32, tag="u2b")
        nc.tensor.matmul(ps_u[:], W1a[:], ctm[:], start=True, stop=False)
        nc.tensor.matmul(ps_u[:], b1r[:], ones_1x16[:], start=False, stop=True)
        nc.scalar.copy(U2b[:], ps_u[:])
        # mask16 [16, N]: relu(q - 2 c.p + c2 - THR)  (0 iff valid)
        for ci in range(NCHUNK):
            sl = slice(ci * CHUNK, (ci + 1) * CHUNK)
            ps_m = sc_psum.tile([MC, CHUNK], F32, tag="m16")
            nc.tensor.matmul(ps_m[:], ones_3x16[:], pTsq[:, sl], start=True,
                             stop=False)
            nc.tensor.matmul(ps_m[:], ctm2[:], pT[:, sl], start=False,
                             stop=True)
            nc.scalar.activation(mask16[:, sl], ps_m[:], ACTF.Relu,
                                 bias=negthr[:], scale=1.0)
        sc_es.close()

        if debug:
            nc.sync.dma_start(d_dbg_mask.ap(), mask16[:])

        # ================= per-cluster MLP + masked max =================
        mlp_es = ExitStack()
        mlp_psum = mlp_es.enter_context(
            tc.tile_pool(name="mlp_psum", bufs=2, space="PSUM"))
        h1_pool = mlp_es.enter_context(tc.tile_pool(name="h1", bufs=6))
        gp_pool = mlp_es.enter_context(tc.tile_pool(name="gp", bufs=2))
        for j in range(MC):
            gparts = gp_pool.tile([C, 2], F32, tag="gparts")
            for grp in range(2):
                ps2 = mlp_psum.tile([C, GRP * CHUNK], F32, tag="ps2")
                h1list = []
                for q in range(GRP):
                    ci = grp * GRP + q
                    sl = slice(ci * CHUNK, (ci + 1) * CHUNK)
                    h1 = h1_pool.tile([C, CHUNK], F32R, tag="h1")
                    if q % 4 == 3:
                        nc.vector.tensor_scalar(h1[:], P3[:, sl],
                                                U2b[:, j:j + 1], 0.0,
                                                op0=ALU.add, op1=ALU.max)
                    else:
                        nc.scalar.activation(h1[:], P3[:, sl], ACTF.Relu,
                                             bias=U2b[:, j:j + 1], scale=1.0)
                    h1list.append(h1)
                for q in range(GRP):
                    qsl = slice(q * CHUNK, (q + 1) * CHUNK)
                    nc.tensor.matmul(ps2[:, qsl], W2r[:], h1list[q][:],
                                     start=True, stop=False)
                for q in range(GRP):
                    ci = grp * GRP + q
                    sl = slice(ci * CHUNK, (ci + 1) * CHUNK)
                    qsl = slice(q * CHUNK, (q + 1) * CHUNK)
                    nc.tensor.matmul(ps2[:, qsl],
                                     NB16[:, j * 128:(j + 1) * 128],
                                     mask16[:, sl], start=False,
                                     stop=True)
                nc.vector.tensor_reduce(gparts[:, grp:grp + 1], ps2[:],
                                        axis=AX.X, op=ALU.max)
            nc.vector.tensor_reduce(G[:, j:j + 1], gparts[:], axis=AX.X,
                                    op=ALU.max)
        mlp_es.close()

        if debug:
            nc.sync.dma_start(d_dbg_G.ap(), G[:])

        # ================= box MLP =================
        bx_es = ExitStack()
        bx_psum = bx_es.enter_context(
            tc.tile_pool(name="bx_psum", bufs=1, space="PSUM"))
        bx_pool = bx_es.enter_context(tc.tile_pool(name="bx", bufs=2))
        # g = relu(gmax + b2)
        nc.scalar.activation(Grelu[:], G[:], ACTF.Relu, bias=b2c[:], scale=1.0)
        ps_g3 = bx_psum.tile([C, MC], F32, tag="g3")
        nc.tensor.matmul(ps_g3[:], W3r[:], Grelu[:], start=True, stop=True)
        g3 = bx_pool.tile([C, MC], F32R, tag="g3s")
        nc.scalar.activation(g3[:], ps_g3[:], ACTF.Relu, bias=b3c[:],
                             scale=1.0)
        ps_g4 = bx_psum.tile([C, MC], F32, tag="g4")
        nc.tensor.matmul(ps_g4[:], W4r[:], g3[:], start=True, stop=True)
        g4 = bx_pool.tile([C, MC], F32R, tag="g4s")
        nc.scalar.activation(g4[:], ps_g4[:], ACTF.Relu, bias=b4c[:],
                             scale=1.0)
        ps_bx = bx_psum.tile([7, MC], F32, tag="bx")
        nc.tensor.matmul(ps_bx[:], Wfr[:], g4[:], start=True, stop=False)
        nc.tensor.matmul(ps_bx[:], bfr[:], ones_1x16[:], start=False,
                         stop=True)
        nc.scalar.copy(boxesT[:], ps_bx[:])
        bx_es.close()

        if debug:
            nc.sync.dma_start(d_dbg_boxesT.ap(), boxesT[:])

        # ================= AllGather box logits =================
        dram = es.enter_context(tc.tile_pool(name="dram", bufs=1, space="DRAM"))
        bounce_in = dram.tile([7, MC], F32)
        bounce_out = dram.tile([NCORES, 7 * MC], F32)
        nc.sync.dma_start(bounce_in[:], boxesT[:])
        nc.gpsimd.collective_compute(
            "AllGather", mybir.AluOpType.bypass,
            replica_groups=[list(range(NCORES))],
            ins=[bounce_in[:].opt()],
            outs=[bounce_out[:].opt()],
        )
        # reassemble: cluster m = 8j+k -> bounce_out[k, c*16+j]
        BTall = cp.tile([7, 128], F32)
        nc.sync.dma_start(
            BTall[:].rearrange("c (j k) -> c j k", k=NCORES),
            bounce_out[:].rearrange("k (c j) -> c j k", j=MC),
        )

        # ================= NMS =================
        nms_es = ExitStack()
        nms_psum = nms_es.enter_context(
            tc.tile_pool(name="nms_psum", bufs=1, space="PSUM"))
        # S7 = sigmoid(logits); BX = [sig | logits] transposed
        nc.scalar.activation(S14[0:7, :], BTall[:], ACTF.Sigmoid)
        ps_bxall = nms_psum.tile([128, 14], F32, tag="bxall")
        nc.tensor.transpose(ps_bxall[:, 0:7], S14[0:7, :], ident[0:7, 0:7])
        nc.tensor.transpose(ps_bxall[:, 7:14], BTall[:], ident[0:7, 0:7])
        nc.vector.tensor_copy(BX[:], ps_bxall[:])
        # cols of BX: 0 score-sig, 1..3 center, 4..6 dims, 7 score-logit
        # lo = c - 0.5 d ; hi = c + 0.5 d
        nc.vector.scalar_tensor_tensor(lo3[:], BX[:, 4:7], -0.5, BX[:, 1:4],
                                       op0=ALU.mult, op1=ALU.add)
        nc.vector.scalar_tensor_tensor(hi3[:], BX[:, 4:7], 0.5, BX[:, 1:4],
                                       op0=ALU.mult, op1=ALU.add)
        nc.vector.tensor_mul(vol[:], BX[:, 4:5], BX[:, 5:6])
        nc.vector.tensor_mul(vol[:], vol[:], BX[:, 6:7])
        # PR = [lo3 | hi3 | vol | score-logit]
        nc.vector.tensor_copy(PR[:, 0:3], lo3[:])
        nc.vector.tensor_copy(PR[:, 3:6], hi3[:])
        nc.vector.tensor_copy(PR[:, 6:7], vol[:])
        nc.vector.tensor_copy(PR[:, 7:8], BX[:, 7:8])
        ps_tp = nms_psum.tile([8, 128], F32, tag="tp")
        nc.tensor.transpose(ps_tp[:], PR[:], ident[:])
        nc.vector.tensor_copy(TPs[:], ps_tp[:])
        # broadcast all 8 rows: psumB[:, r*128:(r+1)*128] = row r over parts
        psB = nms_psum.tile([128, 8 * 128], F32, tag="psB")
        for r in range(8):
            nc.tensor.matmul(psB[:, r * 128:(r + 1) * 128],
                             ER[:, r * 128:(r + 1) * 128],
                             TPs[:], start=True, stop=True)

        def colB(r):
            return psB[:, r * 128:(r + 1) * 128]

        wrk = nms_es.enter_context(tc.tile_pool(name="nms_wrk", bufs=1))
        inter = wrk.tile([128, 128], F32, tag="inter")
        tmpA = wrk.tile([128, 128], F32, tag="tmpA")
        tmpB = wrk.tile([128, 128], F32, tag="tmpB")
        for c in range(3):
            # min(hi_i, hi_j)
            nc.vector.tensor_scalar(tmpA[:], colB(3 + c), hi3[:, c:c + 1],
                                    None, op0=ALU.min)
            # max(lo_i, lo_j)
            nc.vector.tensor_scalar(tmpB[:], colB(c), lo3[:, c:c + 1], None,
                                    op0=ALU.max)
            # w = relu(minhi - maxlo)
            nc.vector.scalar_tensor_tensor(tmpA[:], tmpB[:], -1.0, tmpA[:],
                                           op0=ALU.mult, op1=ALU.add)
            nc.vector.tensor_scalar_max(tmpA[:], tmpA[:], 0.0)
            if c == 0:
                nc.vector.tensor_copy(inter[:], tmpA[:])
            else:
                nc.vector.tensor_mul(inter[:], inter[:], tmpA[:])
        # volsum = vol_i + vol_j + 1e-8
        nc.vector.tensor_scalar(tmpB[:], colB(6), vol[:], 1e-8, op0=ALU.add,
                                op1=ALU.add)
        # D = volsum - inter
        nc.vector.scalar_tensor_tensor(tmpB[:], inter[:], -1.0, tmpB[:],
                                       op0=ALU.mult, op1=ALU.add)
        # P_iou = (4*inter > D)
        nc.vector.scalar_tensor_tensor(tmpA[:], inter[:], 1.0 / NMS_THR,
                                       tmpB[:], op0=ALU.mult, op1=ALU.is_gt)
        # P_score[i,j] = score_j < score_i
        nc.vector.tensor_scalar(tmpB[:], colB(7), BX[:, 7:8], None,
                                op0=ALU.is_lt)
        nc.vector.tensor_mul(P_s[:], tmpA[:], tmpB[:])
        # Jacobi fixpoint: keep_j = !any_i P[i,j] keep_i
        nc.vector.memset(keep[:], 1.0)
        ps_k = nms_psum.tile([128, 1], F32, tag="kps")
        for it in range(NMS_ITERS):
            nc.tensor.matmul(ps_k[:], P_s[:], keep[:], start=True, stop=True)
            nc.vector.tensor_scalar(keep[:], ps_k[:], 0.5, None, op0=ALU.is_lt)
        if debug:
            nc.sync.dma_start(d_dbg_keep.ap(), keep[:])
        # out = coords * keep
        nc.vector.tensor_scalar(outt[:], BX[:, 1:7], keep[:], None,
                                op0=ALU.mult)
        nc.sync.dma_start(d_out.ap(), outt[:])

        nms_es.close()
        es.close()

    nc.compile()
    return nc


def _prep_inputs(vote_points, vote_features, W1, b1, W2, b2, W3, b3, W4, b4,
                 Wf, bf):
    """Pure layout transforms of the full inputs -> per-core input maps."""
    f32 = np.float32
    pts = np.ascontiguousarray(vote_points, dtype=f32)
    feat = np.ascontiguousarray(vote_features, dtype=f32)
    base = {
        "pts96": pts.reshape(128, 96).copy(),
        "pT": pts.T.copy(),
        "featT": feat.T.copy(),
        "W1a": np.ascontiguousarray(W1[:3], f32),
        "W1b": np.ascontiguousarray(W1[3:], f32),
        "W2": np.ascontiguousarray(W2, f32),
        "W3": np.ascontiguousarray(W3, f32),
        "W4": np.ascontiguousarray(W4, f32),
        "Wf": np.ascontiguousarray(Wf, f32),
        "b1r": np.ascontiguousarray(b1, f32).reshape(1, C),
        "b2c": np.ascontiguousarray(b2, f32).reshape(C, 1),
        "b3c": np.ascontiguousarray(b3, f32).reshape(C, 1),
        "b4c": np.ascontiguousarray(b4, f32).reshape(C, 1),
        "bfr": np.ascontiguousarray(bf, f32).reshape(1, 7),
    }
    in_maps = []
    for k in range(NCORES):
        m = dict(base)
        sel = np.zeros((128, MC), f32)
        for j in range(MC):
            sel[NCORES * j + k, j] = 1.0
        m["sel16"] = sel
        in_maps.append(m)
    return in_maps


def kernel(**inputs):
    from concourse.bass_utils import run_bass_kernel_spmd

    if "nc" not in _cache:
        _cache["nc"] = _build(debug=False)
    nc = _cache["nc"]
    in_maps = _prep_inputs(**inputs)
    res = run_bass_kernel_spmd(nc, in_maps, core_ids=list(range(NCORES)))
    out = np.asarray(res.results[0]["out"], dtype=np.float32)
    return out


# revision 17
# speedup vs baseline: 1.4612x; 1.0140x over previous
"""Trainium2 Bass kernel for nn_DetectionHead (VoteNet-style detection head).

Self-contained: builds an 8-core SPMD Bass/Tile kernel, shards the M=128
clusters across cores (interleaved mod 8), replicates FPS + NMS, and
AllGathers the per-core box logits for the final NMS pass.

kernel(**inputs) takes the full unsharded inputs and returns the full
[128, 6] output.
"""

import numpy as np

NCORES = 8
N = 4096          # points
C = 128           # feature channels
M = 128           # clusters
MC = M // NCORES  # clusters per core (16)
NJ = 32           # FPS free-dim (N = 128 * NJ)
RADIUS = 0.5
THR = RADIUS * RADIUS   # 0.25 (d2 < THR)
NMS_THR = 0.25
BIG = 1.0e7
NMS_ITERS = 6
CHUNK = 512
NCHUNK = N // CHUNK       # 8
GRP = 4                   # psum2 groups of 4 chunks -> [128, 2048]

_cache = {}


def _build(debug=False):
    import concourse.bacc as bacc
    import concourse.tile as tile
    import concourse.mybir as mybir
    import concourse.bass_isa as bass_isa

    F32 = mybir.dt.float32
    F32R = mybir.dt.float32r
    BF16 = mybir.dt.bfloat16
    I32 = mybir.dt.int32
    ALU = mybir.AluOpType
    ACTF = mybir.ActivationFunctionType
    AX = mybir.AxisListType

    nc = bacc.Bacc("TRN2", target_bir_lowering=False, debug=False,
                   num_devices=NCORES)

    # ---- DRAM I/O ----
    d_pts96 = nc.dram_tensor("pts96", [128, 96], F32, kind="ExternalInput")
    d_pT = nc.dram_tensor("pT", [3, N], F32, kind="ExternalInput")
    d_featT = nc.dram_tensor("featT", [C, N], F32, kind="ExternalInput")
    d_W1a = nc.dram_tensor("W1a", [3, C], F32, kind="ExternalInput")
    d_W1b = nc.dram_tensor("W1b", [C, C], F32, kind="ExternalInput")
    d_W2 = nc.dram_tensor("W2", [C, C], F32, kind="ExternalInput")
    d_W3 = nc.dram_tensor("W3", [C, C], F32, kind="ExternalInput")
    d_W4 = nc.dram_tensor("W4", [C, C], F32, kind="ExternalInput")
    d_Wf = nc.dram_tensor("Wf", [C, 7], F32, kind="ExternalInput")
    d_b1r = nc.dram_tensor("b1r", [1, C], F32, kind="ExternalInput")
    d_b2c = nc.dram_tensor("b2c", [C, 1], F32, kind="ExternalInput")
    d_b3c = nc.dram_tensor("b3c", [C, 1], F32, kind="ExternalInput")
    d_b4c = nc.dram_tensor("b4c", [C, 1], F32, kind="ExternalInput")
    d_bfr = nc.dram_tensor("bfr", [1, 7], F32, kind="ExternalInput")
    d_sel16 = nc.dram_tensor("sel16", [128, MC], F32, kind="ExternalInput")

    d_out = nc.dram_tensor("out", [M, 6], F32, kind="ExternalOutput")
    if debug:
        d_dbg_centers = nc.dram_tensor("dbg_centers", [M, 3], F32,
                                       kind="ExternalOutput")
        d_dbg_G = nc.dram_tensor("dbg_G", [C, MC], F32, kind="ExternalOutput")
        d_dbg_boxesT = nc.dram_tensor("dbg_boxesT", [7, MC], F32,
                                      kind="ExternalOutput")
        d_dbg_mask = nc.dram_tensor("dbg_mask", [MC, N], F32,
                                    kind="ExternalOutput")
        d_dbg_keep = nc.dram_tensor("dbg_keep", [M, 1], F32,
                                    kind="ExternalOutput")

    from contextlib import ExitStack
    es = ExitStack()
    with tile.TileContext(nc) as tc:
        cp = es.enter_context(tc.tile_pool(name="const", bufs=1))
        stage_es = ExitStack()
        stage_pool = stage_es.enter_context(tc.tile_pool(name="stage", bufs=1))
        featT_stage = stage_pool.tile([C, N], F32)
        # ---- constant / persistent tiles ----
        pts96 = cp.tile([128, 96], F32)
        pT = cp.tile([3, N], F32)
        pTsq = cp.tile([3, N], F32)
        featT = cp.tile([C, N], F32R)
        pTr = cp.tile([3, N], F32R)
        P3 = cp.tile([C, N], F32R)
        mask16 = cp.tile([MC, N], BF16)
        W1a = cp.tile([3, C], F32)
        W1am2 = cp.tile([3, C], F32R)
        W1br = cp.tile([C, C], F32R)
        W2r = cp.tile([C, C], F32R)
        W3r = cp.tile([C, C], F32R)
        W4r = cp.tile([C, C], F32R)
        Wfr = cp.tile([C, 7], F32R)
        W1b = cp.tile([C, C], F32)
        W2 = cp.tile([C, C], F32)
        W3 = cp.tile([C, C], F32)
        W4 = cp.tile([C, C], F32)
        Wf = cp.tile([C, 7], F32)
        b1r = cp.tile([1, C], F32)
        b2c = cp.tile([C, 1], F32)
        b3c = cp.tile([C, 1], F32)
        b4c = cp.tile([C, 1], F32)
        bfr = cp.tile([1, 7], F32)
        sel16 = cp.tile([128, MC], F32)
        ident = cp.tile([128, 128], F32)
        ident_i = cp.tile([128, 128], I32)
        ones_1x128 = cp.tile([1, 128], F32)
        ones_1x16 = cp.tile([1, MC], F32)
        ones_3x16 = cp.tile([3, MC], F32)
        negbig = cp.tile([1, C], F32)
        NB16 = cp.tile([MC, MC * 128], BF16)
        NB16_i = cp.tile([MC, MC * 128], I32)
        ER = cp.tile([8, 8 * 128], F32)
        ER_i = cp.tile([8, 8 * 128], I32)
        IDrow = cp.tile([1, 256], F32)
        IDrow_i = cp.tile([1, 256], I32)
        centers = cp.tile([128, 3], F32)
        U2b = cp.tile([C, MC], F32)
        ctm = cp.tile([3, MC], F32)    # centersT_mine
        ctm2 = cp.tile([3, MC], F32)   # -2 * centersT_mine
        cmine = cp.tile([MC, 3], F32)
        cmsq = cp.tile([MC, 3], F32)
        c2m = cp.tile([MC, 1], F32)
        negthr = cp.tile([MC, 1], F32)
        G = cp.tile([C, MC], F32)
        Grelu = cp.tile([C, MC], F32R)
        boxesT = cp.tile([7, MC], F32)
        # FPS state
        min_d = cp.tile([128, NJ], F32)
        d_newt = cp.tile([128, NJ], F32)
        rowmax = cp.tile([128, 1], F32)
        gb = cp.tile([128, 1], F32)
        partials = cp.tile([128, 3], F32)
        selbb = cp.tile([128, 3], F32)
        masked96 = cp.tile([128, 96], F32)
        diff96 = cp.tile([128, 96], F32)
        diffsq = cp.tile([128, 96], F32)
        iotaN_i = cp.tile([128, NJ], I32)
        # NMS tiles
        S14 = cp.tile([14, 128], F32)
        BX = cp.tile([128, 14], F32)
        PR = cp.tile([128, 8], F32)
        TPs = cp.tile([8, 128], F32)
        P_s = cp.tile([128, 128], F32)
        keep = cp.tile([128, 1], F32)
        lo3 = cp.tile([128, 3], F32)
        hi3 = cp.tile([128, 3], F32)
        vol = cp.tile([128, 1], F32)
        outt = cp.tile([128, 6], F32)

        # ---- input DMA ----
        nc.sync.dma_start(pts96[:], d_pts96.ap())
        nc.sync.dma_start(pT[:], d_pT.ap())
        nc.sync.dma_start(featT_stage[:], d_featT.ap())
        nc.sync.dma_start(W1a[:], d_W1a.ap())
        nc.sync.dma_start(W1b[:], d_W1b.ap())
        nc.sync.dma_start(W2[:], d_W2.ap())
        nc.sync.dma_start(W3[:], d_W3.ap())
        nc.sync.dma_start(W4[:], d_W4.ap())
        nc.sync.dma_start(Wf[:], d_Wf.ap())
        nc.sync.dma_start(b1r[:], d_b1r.ap())
        nc.sync.dma_start(b2c[:], d_b2c.ap())
        nc.sync.dma_start(b3c[:], d_b3c.ap())
        nc.sync.dma_start(b4c[:], d_b4c.ap())
        nc.sync.dma_start(bfr[:], d_bfr.ap())
        nc.sync.dma_start(sel16[:], d_sel16.ap())

        # ---- constants ----
        nc.gpsimd.iota(ident_i[:], pattern=[[1, 128]], base=0,
                       channel_multiplier=-1)
        nc.vector.tensor_scalar(ident[:], ident_i[:], 0, None,
                                op0=ALU.is_equal)
        nc.vector.memset(ones_1x128[:], 1.0)
        nc.vector.memset(ones_1x16[:], 1.0)
        nc.vector.memset(ones_3x16[:], 1.0)
        nc.vector.memset(negbig[:], -BIG)
        nc.gpsimd.iota(NB16_i[:].rearrange("p (j c) -> p j c", c=128),
                       pattern=[[1, MC], [0, 128]], base=0,
                       channel_multiplier=-1)
        nc.vector.tensor_scalar(NB16[:], NB16_i[:], 0, -BIG,
                                op0=ALU.is_equal, op1=ALU.mult)
        nc.gpsimd.iota(ER_i[:].rearrange("p (j c) -> p j c", c=128),
                       pattern=[[1, 8], [0, 128]], base=0,
                       channel_multiplier=-1)
        nc.vector.tensor_scalar(ER[:], ER_i[:], 0, None, op0=ALU.is_equal)
        nc.gpsimd.iota(IDrow_i[:], pattern=[[1, 256]], base=0,
                       channel_multiplier=0)
        nc.vector.tensor_scalar(IDrow[:], IDrow_i[:], 127, None,
                                op0=ALU.is_equal)
        nc.vector.tensor_scalar_mul(W1am2[:], W1a[:], -2.0)
        nc.vector.tensor_copy(featT[:], featT_stage[:])
        nc.vector.tensor_copy(W1br[:], W1b[:])
        nc.vector.tensor_copy(W2r[:], W2[:])
        nc.vector.tensor_copy(W3r[:], W3[:])
        nc.vector.tensor_copy(W4r[:], W4[:])
        nc.vector.tensor_copy(Wfr[:], Wf[:])
        nc.scalar.copy(pTr[:], pT[:])
        nc.gpsimd.iota(iotaN_i[:], pattern=[[1, NJ]], base=0,
                       channel_multiplier=NJ)
        nc.vector.tensor_scalar(min_d[:], iotaN_i[:], -1.0, None,
                                op0=ALU.mult)
        nc.vector.tensor_mul(pTsq[:], pT[:], pT[:])

        # views of pts96
        pts_pjc = pts96[:].rearrange("p (j c) -> p j c", c=3)

        stage_es.close()
        # ================= FPS =================
        fps_es = ExitStack()
        fps_psum = fps_es.enter_context(
            tc.tile_pool(name="fps_psum", bufs=2, space="PSUM"))
        p3_psum = fps_es.enter_context(
            tc.tile_pool(name="p3_psum", bufs=1, space="PSUM"))
        ctr_psum = fps_es.enter_context(
            tc.tile_pool(name="ctr_psum", bufs=1, space="PSUM"))
        srow_pool = fps_es.enter_context(tc.tile_pool(name="srow", bufs=3))
        centers_ps = ctr_psum.tile([128, 3], F32)

        # ---- P3 = W1b^T @ featT + (-2 W1a)^T @ pT  (cluster independent) ----
        for ci in range(NCHUNK):
            sl = slice(ci * CHUNK, (ci + 1) * CHUNK)
            ps = p3_psum.tile([C, CHUNK], F32, tag="p3ps")
            nc.tensor.matmul(ps[:], W1br[:], featT[:, sl], start=True,
                             stop=False)
            nc.tensor.matmul(ps[:], W1am2[:], pTr[:, sl], start=False,
                             stop=True)
            nc.scalar.copy(P3[:, sl], ps[:])

        def fps_select_update(t):
            """Select center t from min_d (+pending d_newt), update state."""
            if t == 0:
                pass  # min_d holds -n (selects point 0)
            elif t == 1:
                nc.vector.tensor_copy(min_d[:], d_newt[:])
            else:
                nc.vector.tensor_tensor(min_d[:], min_d[:], d_newt[:],
                                        op=ALU.min)
            nc.vector.tensor_reduce(rowmax[:], min_d[:], axis=AX.X,
                                    op=ALU.max)
            nc.gpsimd.partition_all_reduce(gb[:], rowmax[:], channels=128,
                                           reduce_op=bass_isa.ReduceOp.max)
            # masked96 = (min_d >= gmax) * pts   (global one-hot mask)
            nc.vector.scalar_tensor_tensor(
                out=masked96[:].rearrange("p (j c) -> p c j", c=3),
                in0=min_d[:].unsqueeze(1).broadcast_to([128, 3, NJ]),
                scalar=gb[:],
                in1=pts96[:].rearrange("p (j c) -> p c j", c=3),
                op0=ALU.is_ge, op1=ALU.mult)
            nc.vector.tensor_reduce(
                partials[:], masked96[:].rearrange("p (j c) -> p c j", c=3),
                axis=AX.X, op=ALU.add)
            nc.gpsimd.partition_all_reduce(selbb[:], partials[:], channels=128,
                                           reduce_op=bass_isa.ReduceOp.add)
            # record center: centers_ps += e_t (x) c_t
            srow = srow_pool.tile([1, 3], F32, tag="srow")
            nc.scalar.copy(srow[:], selbb[0:1, :])
            nc.tensor.matmul(centers_ps[:],
                             IDrow[0:1, 127 - t:255 - t], srow[:],
                             start=(t == 0), stop=(t == M - 1),
                             skip_group_check=True)
            if t == M - 1:
                return
            # d_newt = sum_c (pts - c_t)^2
            nc.vector.tensor_tensor(
                diff96[:].rearrange("p (j c) -> p j c", c=3),
                pts96[:].rearrange("p (j c) -> p j c", c=3),
                selbb[:].unsqueeze(1).broadcast_to([128, NJ, 3]),
                op=ALU.subtract)
            nc.vector.tensor_mul(diffsq[:], diff96[:], diff96[:])
            nc.vector.tensor_reduce(
                d_newt[:], diffsq[:].rearrange("p (j c) -> p j c", c=3),
                axis=AX.X, op=ALU.add)

        for t in range(M):
            fps_select_update(t)

        nc.scalar.copy(centers[:], centers_ps[:])
        fps_es.close()

        if debug:
            nc.sync.dma_start(d_dbg_centers.ap(), centers[:])

        # ================= post-FPS per-core prep =================
        sc_es = ExitStack()
        sc_psum = sc_es.enter_context(
            tc.tile_pool(name="sc_psum", bufs=2, space="PSUM"))

        # centers_mine [16, 3]
        ps_cm = sc_psum.tile([MC, 3], F32, tag="cm")
        nc.tensor.matmul(ps_cm[:], sel16[:], centers[:], start=True, stop=True)
        nc.scalar.copy(cmine[:], ps_cm[:])
        # c2 and thresholds
        nc.vector.tensor_mul(cmsq[:], cmine[:], cmine[:])
        nc.vector.tensor_reduce(c2m[:], cmsq[:], axis=AX.X, op=ALU.add)
        # negthr = c2 - THR
        nc.vector.tensor_scalar(negthr[:], c2m[:], -THR, None, op0=ALU.add)
        # centersT_mine [3, 16]
        ps_ctm = sc_psum.tile([3, MC], F32, tag="ctm")
        nc.tensor.transpose(ps_ctm[:], cmine[:], ident[0:MC, 0:MC])
        nc.scalar.copy(ctm[:], ps_ctm[:])
        nc.scalar.mul(ctm2[:], ps_ctm[:], -2.0)
        # U2b [128, 16] = W1a^T @ centersT_mine + b1
        ps_u = sc_psum.tile([C, MC], F32, tag="u2b")
        nc.tensor.matmul(ps_u[:], W1a[:], ctm[:], start=True, stop=False)
        nc.tensor.matmul(ps_u[:], b1r[:], ones_1x16[:], start=False, stop=True)
        nc.scalar.copy(U2b[:], ps_u[:])
        # mask16 [16, N]: relu(q - 2 c.p + c2 - THR)  (0 iff valid)
        for ci in range(NCHUNK):
            sl = slice(ci * CHUNK, (ci + 1) * CHUNK)
            ps_m = sc_psum.tile([MC, CHUNK], F32, tag="m16")
            nc.tensor.matmul(ps_m[:], ones_3x16[:], pTsq[:, sl], start=True,
                             stop=False)
            nc.tensor.matmul(ps_m[:], ctm2[:], pT[:, sl], start=False,
                             stop=True)
            nc.scalar.activation(mask16[:, sl], ps_m[:], ACTF.Relu,
                                 bias=negthr[:], scale=1.0)
        sc_es.close()

        if debug:
            nc.sync.dma_start(d_dbg_mask.ap(), mask16[:])

        # ================= per-cluster MLP + masked max =================
        mlp_es = ExitStack()
        mlp_psum = mlp_es.enter_context(
            tc.tile_pool(name="mlp_psum", bufs=2, space="PSUM"))
        h1_pool = mlp_es.enter_context(tc.tile_pool(name="h1", bufs=6))
        gp_pool = mlp_es.enter_context(tc.tile_pool(name="gp", bufs=2))
        for j in range(MC):
            gparts = gp_pool.tile([C, 2], F32, tag="gparts")
            for grp in range(2):
                ps2 = mlp_psum.tile([C, GRP * CHUNK], F32, tag="ps2")
                h1list = []
                for q in range(GRP):
                    ci = grp * GRP + q
                    sl = slice(ci * CHUNK, (ci + 1) * CHUNK)
                    h1 = h1_pool.tile([C, CHUNK], F32R, tag="h1")
                    if q % 4 == 3:
                        nc.vector.tensor_scalar(h1[:], P3[:, sl],
                                                U2b[:, j:j + 1], 0.0,
                                                op0=ALU.add, op1=ALU.max)
                    else:
                        nc.scalar.activation(h1[:], P3[:, sl], ACTF.Relu,
                                             bias=U2b[:, j:j + 1], scale=1.0)
                    h1list.append(h1)
                for q in range(GRP):
                    qsl = slice(q * CHUNK, (q + 1) * CHUNK)
                    nc.tensor.matmul(ps2[:, qsl], W2r[:], h1list[q][:],
                                     start=True, stop=False)
                for q in range(GRP):
                    ci = grp * GRP + q
                    sl = slice(ci * CHUNK, (ci + 1) * CHUNK)
                    qsl = slice(q * CHUNK, (q + 1) * CHUNK)
                    nc.tensor.matmul(ps2[:, qsl],
                                     NB16[:, j * 128:(j + 1) * 128],
                                     mask16[:, sl], start=False,
                                     stop=True)
                nc.vector.tensor_reduce(gparts[:, grp:grp + 1], ps2[:],
                                        axis=AX.X, op=ALU.max)
            nc.vector.tensor_reduce(G[:, j:j + 1], gparts[:], axis=AX.X,
                                    op=ALU.max)
        mlp_es.close()

        if debug:
            nc.sync.dma_start(d_dbg_G.ap(), G[:])

        # ================= box MLP =================
        bx_es = ExitStack()
        bx_psum = bx_es.enter_context(
            tc.tile_pool(name="bx_psum", bufs=1, space="PSUM"))
        bx_pool = bx_es.enter_context(tc.tile_pool(name="bx", bufs=2))
        # g = relu(gmax + b2)
        nc.scalar.activation(Grelu[:], G[:], ACTF.Relu, bias=b2c[:], scale=1.0)
        ps_g3 = bx_psum.tile([C, MC], F32, tag="g3")
        nc.tensor.matmul(ps_g3[:], W3r[:], Grelu[:], start=True, stop=True)
        g3 = bx_pool.tile([C, MC], F32R, tag="g3s")
        nc.scalar.activation(g3[:], ps_g3[:], ACTF.Relu, bias=b3c[:],
                             scale=1.0)
        ps_g4 = bx_psum.tile([C, MC], F32, tag="g4")
        nc.tensor.matmul(ps_g4[:], W4r[:], g3[:], start=True, stop=True)
        g4 = bx_pool.tile([C, MC], F32R, tag="g4s")
        nc.scalar.activation(g4[:], ps_g4[:], ACTF.Relu, bias=b4c[:],
                             scale=1.0)
        ps_bx = bx_psum.tile([7, MC], F32, tag="bx")
        nc.tensor.matmul(ps_bx[:], Wfr[:], g4[:], start=True, stop=False)
        nc.tensor.matmul(ps_bx[:], bfr[:], ones_1x16[:], start=False,
                         stop=True)
        nc.scalar.copy(boxesT[:], ps_bx[:])
        bx_es.close()

        if debug:
            nc.sync.dma_start(d_dbg_boxesT.ap(), boxesT[:])

        # ================= AllGather box logits =================
        dram = es.enter_context(tc.tile_pool(name="dram", bufs=1, space="DRAM"))
        bounce_in = dram.tile([7, MC], F32)
        bounce_out = dram.tile([NCORES, 7 * MC], F32)
        nc.sync.dma_start(bounce_in[:], boxesT[:])
        nc.gpsimd.collective_compute(
            "AllGather", mybir.AluOpType.bypass,
            replica_groups=[list(range(NCORES))],
            ins=[bounce_in[:].opt()],
            outs=[bounce_out[:].opt()],
        )
        # reassemble: cluster m = 8j+k -> bounce_out[k, c*16+j]
        BTall = cp.tile([7, 128], F32)
        nc.sync.dma_start(
            BTall[:].rearrange("c (j k) -> c j k", k=NCORES),
            bounce_out[:].rearrange("k (c j) -> c j k", j=MC),
        )

        # ================= NMS =================
        nms_es = ExitStack()
        nms_psum = nms_es.enter_context(
            tc.tile_pool(name="nms_psum", bufs=1, space="PSUM"))
        # S7 = sigmoid(logits); BX = [sig | logits] transposed
        nc.scalar.activation(S14[0:7, :], BTall[:], ACTF.Sigmoid)
        ps_bxall = nms_psum.tile([128, 14], F32, tag="bxall")
        nc.tensor.transpose(ps_bxall[:, 0:7], S14[0:7, :], ident[0:7, 0:7])
        nc.tensor.transpose(ps_bxall[:, 7:14], BTall[:], ident[0:7, 0:7])
        nc.vector.tensor_copy(BX[:], ps_bxall[:])
        # cols of BX: 0 score-sig, 1..3 center, 4..6 dims, 7 score-logit
        # lo = c - 0.5 d ; hi = c + 0.5 d
        nc.vector.scalar_tensor_tensor(lo3[:], BX[:, 4:7], -0.5, BX[:, 1:4],
                                       op0=ALU.mult, op1=ALU.add)
        nc.vector.scalar_tensor_tensor(hi3[:], BX[:, 4:7], 0.5, BX[:, 1:4],
                                       op0=ALU.mult, op1=ALU.add)
        nc.vector.tensor_mul(vol[:], BX[:, 4:5], BX[:, 5:6])
        nc.vector.tensor_mul(vol[:], vol[:], BX[:, 6:7])
        # PR = [lo3 | hi3 | vol | score-logit]
        nc.vector.tensor_copy(PR[:, 0:3], lo3[:])
        nc.vector.tensor_copy(PR[:, 3:6], hi3[:])
        nc.vector.tensor_copy(PR[:, 6:7], vol[:])
        nc.vector.tensor_copy(PR[:, 7:8], BX[:, 7:8])
        ps_tp = nms_psum.tile([8, 128], F32, tag="tp")
        nc.tensor.transpose(ps_tp[:], PR[:], ident[:])
        nc.vector.tensor_copy(TPs[:], ps_tp[:])
        # broadcast all 8 rows: psumB[:, r*128:(r+1)*128] = row r over parts
        psB = nms_psum.tile([128, 8 * 128], F32, tag="psB")
        for r in range(8):
            nc.tensor.matmul(psB[:, r * 128:(r + 1) * 128],
                             ER[:, r * 128:(r + 1) * 128],
                             TPs[:], start=True, stop=True)

        def colB(r):
            return psB[:, r * 128:(r + 1) * 128]

        wrk = nms_es.enter_context(tc.tile_pool(name="nms_wrk", bufs=1))
        inter = wrk.tile([128, 128], F32, tag="inter")
        tmpA = wrk.tile([128, 128], F32, tag="tmpA")
        tmpB = wrk.tile([128, 128], F32, tag="tmpB")
        for c in range(3):
            # min(hi_i, hi_j)
            nc.vector.tensor_scalar(tmpA[:], colB(3 + c), hi3[:, c:c + 1],
                                    None, op0=ALU.min)
            # max(lo_i, lo_j)
            nc.vector.tensor_scalar(tmpB[:], colB(c), lo3[:, c:c + 1], None,
                                    op0=ALU.max)
            # w = relu(minhi - maxlo)
            nc.vector.scalar_tensor_tensor(tmpA[:], tmpB[:], -1.0, tmpA[:],
                                           op0=ALU.mult, op1=ALU.add)
            nc.vector.tensor_scalar_max(tmpA[:], tmpA[:], 0.0)
            if c == 0:
                nc.vector.tensor_copy(inter[:], tmpA[:])
            else:
                nc.vector.tensor_mul(inter[:], inter[:], tmpA[:])
        # volsum = vol_i + vol_j + 1e-8
        nc.vector.tensor_scalar(tmpB[:], colB(6), vol[:], 1e-8, op0=ALU.add,
                                op1=ALU.add)
        # D = volsum - inter
        nc.vector.scalar_tensor_tensor(tmpB[:], inter[:], -1.0, tmpB[:],
                                       op0=ALU.mult, op1=ALU.add)
        # P_iou = (4*inter > D)
        nc.vector.scalar_tensor_tensor(tmpA[:], inter[:], 1.0 / NMS_THR,
                                       tmpB[:], op0=ALU.mult, op1=ALU.is_gt)
        # P_score[i,j] = score_j < score_i
        nc.vector.tensor_scalar(tmpB[:], colB(7), BX[:, 7:8], None,
                                op0=ALU.is_lt)
        nc.vector.tensor_mul(P_s[:], tmpA[:], tmpB[:])
        # Jacobi fixpoint: keep_j = !any_i P[i,j] keep_i
        nc.vector.memset(keep[:], 1.0)
        ps_k = nms_psum.tile([128, 1], F32, tag="kps")
        for it in range(NMS_ITERS):
            nc.tensor.matmul(ps_k[:], P_s[:], keep[:], start=True, stop=True)
            nc.vector.tensor_scalar(keep[:], ps_k[:], 0.5, None, op0=ALU.is_lt)
        if debug:
            nc.sync.dma_start(d_dbg_keep.ap(), keep[:])
        # out = coords * keep
        nc.vector.tensor_scalar(outt[:], BX[:, 1:7], keep[:], None,
                                op0=ALU.mult)
        nc.sync.dma_start(d_out.ap(), outt[:])

        nms_es.close()
        es.close()

    nc.compile()
    return nc


def _prep_inputs(vote_points, vote_features, W1, b1, W2, b2, W3, b3, W4, b4,
                 Wf, bf):
    """Pure layout transforms of the full inputs -> per-core input maps."""
    f32 = np.float32
    pts = np.ascontiguousarray(vote_points, dtype=f32)
    feat = np.ascontiguousarray(vote_features, dtype=f32)
    base = {
        "pts96": pts.reshape(128, 96).copy(),
        "pT": pts.T.copy(),
        "featT": feat.T.copy(),
        "W1a": np.ascontiguousarray(W1[:3], f32),
        "W1b": np.ascontiguousarray(W1[3:], f32),
        "W2": np.ascontiguousarray(W2, f32),
        "W3": np.ascontiguousarray(W3, f32),
        "W4": np.ascontiguousarray(W4, f32),
        "Wf": np.ascontiguousarray(Wf, f32),
        "b1r": np.ascontiguousarray(b1, f32).reshape(1, C),
        "b2c": np.ascontiguousarray(b2, f32).reshape(C, 1),
        "b3c": np.ascontiguousarray(b3, f32).reshape(C, 1),
        "b4c": np.ascontiguousarray(b4, f32).reshape(C, 1),
        "bfr": np.ascontiguousarray(bf, f32).reshape(1, 7),
    }
    in_maps = []
    for k in range(NCORES):
        m = dict(base)
        sel = np.zeros((128, MC), f32)
        for j in range(MC):
            sel[NCORES * j + k, j] = 1.0
        m["sel16"] = sel
        in_maps.append(m)
    return in_maps


def kernel(**inputs):
    from concourse.bass_utils import run_bass_kernel_spmd

    if "nc" not in _cache:
        _cache["nc"] = _build(debug=False)
    nc = _cache["nc"]
    in_maps = _prep_inputs(**inputs)
    res = run_bass_kernel_spmd(nc, in_maps, core_ids=list(range(NCORES)))
    out = np.asarray(res.results[0]["out"], dtype=np.float32)
    return out


# revision 18
# speedup vs baseline: 1.4617x; 1.0003x over previous
"""Trainium2 Bass kernel for nn_DetectionHead (VoteNet-style detection head).

Self-contained: builds an 8-core SPMD Bass/Tile kernel, shards the M=128
clusters across cores (interleaved mod 8), replicates FPS + NMS, and
AllGathers the per-core box logits for the final NMS pass.

kernel(**inputs) takes the full unsharded inputs and returns the full
[128, 6] output.
"""

import numpy as np

NCORES = 8
N = 4096          # points
C = 128           # feature channels
M = 128           # clusters
MC = M // NCORES  # clusters per core (16)
NJ = 32           # FPS free-dim (N = 128 * NJ)
RADIUS = 0.5
THR = RADIUS * RADIUS   # 0.25 (d2 < THR)
NMS_THR = 0.25
BIG = 1.0e7
NMS_ITERS = 6
CHUNK = 512
NCHUNK = N // CHUNK       # 8
GRP = 4                   # psum2 groups of 4 chunks -> [128, 2048]

_cache = {}


def _build(debug=False):
    import concourse.bacc as bacc
    import concourse.tile as tile
    import concourse.mybir as mybir
    import concourse.bass_isa as bass_isa

    F32 = mybir.dt.float32
    F32R = mybir.dt.float32r
    BF16 = mybir.dt.bfloat16
    I32 = mybir.dt.int32
    ALU = mybir.AluOpType
    ACTF = mybir.ActivationFunctionType
    AX = mybir.AxisListType

    nc = bacc.Bacc("TRN2", target_bir_lowering=False, debug=False,
                   num_devices=NCORES)

    # ---- DRAM I/O ----
    d_pts96 = nc.dram_tensor("pts96", [128, 96], F32, kind="ExternalInput")
    d_pT = nc.dram_tensor("pT", [3, N], F32, kind="ExternalInput")
    d_featT = nc.dram_tensor("featT", [C, N], F32, kind="ExternalInput")
    d_W1a = nc.dram_tensor("W1a", [3, C], F32, kind="ExternalInput")
    d_W1b = nc.dram_tensor("W1b", [C, C], F32, kind="ExternalInput")
    d_W2 = nc.dram_tensor("W2", [C, C], F32, kind="ExternalInput")
    d_W3 = nc.dram_tensor("W3", [C, C], F32, kind="ExternalInput")
    d_W4 = nc.dram_tensor("W4", [C, C], F32, kind="ExternalInput")
    d_Wf = nc.dram_tensor("Wf", [C, 7], F32, kind="ExternalInput")
    d_b1r = nc.dram_tensor("b1r", [1, C], F32, kind="ExternalInput")
    d_b2c = nc.dram_tensor("b2c", [C, 1], F32, kind="ExternalInput")
    d_b3c = nc.dram_tensor("b3c", [C, 1], F32, kind="ExternalInput")
    d_b4c = nc.dram_tensor("b4c", [C, 1], F32, kind="ExternalInput")
    d_bfr = nc.dram_tensor("bfr", [1, 7], F32, kind="ExternalInput")
    d_sel16 = nc.dram_tensor("sel16", [128, MC], F32, kind="ExternalInput")

    d_out = nc.dram_tensor("out", [M, 6], F32, kind="ExternalOutput")
    if debug:
        d_dbg_centers = nc.dram_tensor("dbg_centers", [M, 3], F32,
                                       kind="ExternalOutput")
        d_dbg_G = nc.dram_tensor("dbg_G", [C, MC], F32, kind="ExternalOutput")
        d_dbg_boxesT = nc.dram_tensor("dbg_boxesT", [7, MC], F32,
                                      kind="ExternalOutput")
        d_dbg_mask = nc.dram_tensor("dbg_mask", [MC, N], F32,
                                    kind="ExternalOutput")
        d_dbg_keep = nc.dram_tensor("dbg_keep", [M, 1], F32,
                                    kind="ExternalOutput")

    from contextlib import ExitStack
    es = ExitStack()
    with tile.TileContext(nc) as tc:
        cp = es.enter_context(tc.tile_pool(name="const", bufs=1))
        stage_es = ExitStack()
        stage_pool = stage_es.enter_context(tc.tile_pool(name="stage", bufs=1))
        featT_stage = stage_pool.tile([C, N], F32)
        # ---- constant / persistent tiles ----
        pts96 = cp.tile([128, 96], F32)
        pT = cp.tile([3, N], F32)
        pTsq = cp.tile([3, N], F32)
        featT = cp.tile([C, N], F32R)
        pTr = cp.tile([3, N], F32R)
        P3 = cp.tile([C, N], F32R)
        mask16 = cp.tile([MC, N], BF16)
        W1a = cp.tile([3, C], F32)
        W1am2 = cp.tile([3, C], F32R)
        W1br = cp.tile([C, C], F32R)
        W2r = cp.tile([C, C], F32R)
        W3r = cp.tile([C, C], F32R)
        W4r = cp.tile([C, C], F32R)
        Wfr = cp.tile([C, 7], F32R)
        W1b = cp.tile([C, C], F32)
        W2 = cp.tile([C, C], F32)
        W3 = cp.tile([C, C], F32)
        W4 = cp.tile([C, C], F32)
        Wf = cp.tile([C, 7], F32)
        b1r = cp.tile([1, C], F32)
        b2c = cp.tile([C, 1], F32)
        b3c = cp.tile([C, 1], F32)
        b4c = cp.tile([C, 1], F32)
        bfr = cp.tile([1, 7], F32)
        sel16 = cp.tile([128, MC], F32)
        ident = cp.tile([128, 128], F32)
        ident_i = cp.tile([128, 128], I32)
        ones_1x128 = cp.tile([1, 128], F32)
        ones_1x16 = cp.tile([1, MC], F32)
        ones_3x16 = cp.tile([3, MC], F32)
        negbig = cp.tile([1, C], F32)
        NB16 = cp.tile([MC, MC * 128], BF16)
        NB16_i = cp.tile([MC, MC * 128], I32)
        ER = cp.tile([8, 8 * 128], F32)
        ER_i = cp.tile([8, 8 * 128], I32)
        IDrow = cp.tile([1, 256], F32)
        IDrow_i = cp.tile([1, 256], I32)
        centers = cp.tile([128, 3], F32)
        U2b = cp.tile([C, MC], F32)
        ctm = cp.tile([3, MC], F32)    # centersT_mine
        ctm2 = cp.tile([3, MC], F32)   # -2 * centersT_mine
        cmine = cp.tile([MC, 3], F32)
        cmsq = cp.tile([MC, 3], F32)
        c2m = cp.tile([MC, 1], F32)
        negthr = cp.tile([MC, 1], F32)
        G = cp.tile([C, MC], F32)
        Grelu = cp.tile([C, MC], F32R)
        boxesT = cp.tile([7, MC], F32)
        # FPS state
        min_d = cp.tile([128, NJ], F32)
        d_newt = cp.tile([128, NJ], F32)
        rowmax = cp.tile([128, 1], F32)
        gb = cp.tile([128, 1], F32)
        partials = cp.tile([128, 3], F32)
        selbb = cp.tile([128, 3], F32)
        masked96 = cp.tile([128, 96], F32)
        diff96 = cp.tile([128, 96], F32)
        diffsq = cp.tile([128, 96], F32)
        iotaN_i = cp.tile([128, NJ], I32)
        # NMS tiles
        S14 = cp.tile([14, 128], F32)
        BX = cp.tile([128, 14], F32)
        PR = cp.tile([128, 8], F32)
        TPs = cp.tile([8, 128], F32)
        P_s = cp.tile([128, 128], F32)
        keep = cp.tile([128, 1], F32)
        lo3 = cp.tile([128, 3], F32)
        hi3 = cp.tile([128, 3], F32)
        vol = cp.tile([128, 1], F32)
        outt = cp.tile([128, 6], F32)

        # ---- input DMA ----
        nc.sync.dma_start(pts96[:], d_pts96.ap())
        nc.sync.dma_start(pT[:], d_pT.ap())
        nc.sync.dma_start(featT_stage[:], d_featT.ap())
        nc.sync.dma_start(W1a[:], d_W1a.ap())
        nc.sync.dma_start(W1b[:], d_W1b.ap())
        nc.sync.dma_start(W2[:], d_W2.ap())
        nc.sync.dma_start(W3[:], d_W3.ap())
        nc.sync.dma_start(W4[:], d_W4.ap())
        nc.sync.dma_start(Wf[:], d_Wf.ap())
        nc.sync.dma_start(b1r[:], d_b1r.ap())
        nc.sync.dma_start(b2c[:], d_b2c.ap())
        nc.sync.dma_start(b3c[:], d_b3c.ap())
        nc.sync.dma_start(b4c[:], d_b4c.ap())
        nc.sync.dma_start(bfr[:], d_bfr.ap())
        nc.sync.dma_start(sel16[:], d_sel16.ap())

        # ---- constants ----
        nc.gpsimd.iota(ident_i[:], pattern=[[1, 128]], base=0,
                       channel_multiplier=-1)
        nc.vector.tensor_scalar(ident[:], ident_i[:], 0, None,
                                op0=ALU.is_equal)
        nc.vector.memset(ones_1x128[:], 1.0)
        nc.vector.memset(ones_1x16[:], 1.0)
        nc.vector.memset(ones_3x16[:], 1.0)
        nc.vector.memset(negbig[:], -BIG)
        nc.gpsimd.iota(NB16_i[:].rearrange("p (j c) -> p j c", c=128),
                       pattern=[[1, MC], [0, 128]], base=0,
                       channel_multiplier=-1)
        nc.vector.tensor_scalar(NB16[:], NB16_i[:], 0, -BIG,
                                op0=ALU.is_equal, op1=ALU.mult)
        nc.gpsimd.iota(ER_i[:].rearrange("p (j c) -> p j c", c=128),
                       pattern=[[1, 8], [0, 128]], base=0,
                       channel_multiplier=-1)
        nc.vector.tensor_scalar(ER[:], ER_i[:], 0, None, op0=ALU.is_equal)
        nc.gpsimd.iota(IDrow_i[:], pattern=[[1, 256]], base=0,
                       channel_multiplier=0)
        nc.vector.tensor_scalar(IDrow[:], IDrow_i[:], 127, None,
                                op0=ALU.is_equal)
        nc.vector.tensor_scalar_mul(W1am2[:], W1a[:], -2.0)
        nc.vector.tensor_copy(featT[:], featT_stage[:])
        nc.vector.tensor_copy(W1br[:], W1b[:])
        nc.vector.tensor_copy(W2r[:], W2[:])
        nc.vector.tensor_copy(W3r[:], W3[:])
        nc.vector.tensor_copy(W4r[:], W4[:])
        nc.vector.tensor_copy(Wfr[:], Wf[:])
        nc.scalar.copy(pTr[:], pT[:])
        nc.gpsimd.iota(iotaN_i[:], pattern=[[1, NJ]], base=0,
                       channel_multiplier=NJ)
        nc.vector.tensor_scalar(min_d[:], iotaN_i[:], -1.0, None,
                                op0=ALU.mult)
        nc.vector.tensor_mul(pTsq[:], pT[:], pT[:])

        # views of pts96
        pts_pjc = pts96[:].rearrange("p (j c) -> p j c", c=3)

        stage_es.close()
        # ================= FPS =================
        fps_es = ExitStack()
        fps_psum = fps_es.enter_context(
            tc.tile_pool(name="fps_psum", bufs=2, space="PSUM"))
        p3_psum = fps_es.enter_context(
            tc.tile_pool(name="p3_psum", bufs=1, space="PSUM"))
        ctr_psum = fps_es.enter_context(
            tc.tile_pool(name="ctr_psum", bufs=1, space="PSUM"))
        srow_pool = fps_es.enter_context(tc.tile_pool(name="srow", bufs=3))
        centers_ps = ctr_psum.tile([128, 3], F32)

        # ---- P3 = W1b^T @ featT + (-2 W1a)^T @ pT  (cluster independent) ----
        for ci in range(NCHUNK):
            sl = slice(ci * CHUNK, (ci + 1) * CHUNK)
            ps = p3_psum.tile([C, CHUNK], F32, tag="p3ps")
            nc.tensor.matmul(ps[:], W1br[:], featT[:, sl], start=True,
                             stop=False)
            nc.tensor.matmul(ps[:], W1am2[:], pTr[:, sl], start=False,
                             stop=True)
            nc.scalar.copy(P3[:, sl], ps[:])

        def fps_select_update(t):
            """Select center t from min_d (+pending d_newt), update state."""
            if t == 0:
                pass  # min_d holds -n (selects point 0)
            elif t == 1:
                nc.vector.tensor_copy(min_d[:], d_newt[:])
            else:
                nc.vector.tensor_tensor(min_d[:], min_d[:], d_newt[:],
                                        op=ALU.min)
            nc.vector.tensor_reduce(rowmax[:], min_d[:], axis=AX.X,
                                    op=ALU.max)
            nc.gpsimd.partition_all_reduce(gb[:], rowmax[:], channels=128,
                                           reduce_op=bass_isa.ReduceOp.max)
            # masked96 = (min_d >= gmax) * pts   (global one-hot mask)
            nc.vector.scalar_tensor_tensor(
                out=masked96[:].rearrange("p (j c) -> p c j", c=3),
                in0=min_d[:].unsqueeze(1).broadcast_to([128, 3, NJ]),
                scalar=gb[:],
                in1=pts96[:].rearrange("p (j c) -> p c j", c=3),
                op0=ALU.is_ge, op1=ALU.mult)
            nc.vector.tensor_reduce(
                partials[:], masked96[:].rearrange("p (j c) -> p c j", c=3),
                axis=AX.X, op=ALU.add)
            nc.gpsimd.partition_all_reduce(selbb[:], partials[:], channels=128,
                                           reduce_op=bass_isa.ReduceOp.add)
            # record center: centers_ps += e_t (x) c_t
            srow = srow_pool.tile([1, 3], F32, tag="srow")
            nc.scalar.copy(srow[:], selbb[0:1, :])
            nc.tensor.matmul(centers_ps[:],
                             IDrow[0:1, 127 - t:255 - t], srow[:],
                             start=(t == 0), stop=(t == M - 1),
                             skip_group_check=True)
            if t == M - 1:
                return
            # d_newt = sum_c (pts - c_t)^2
            nc.vector.tensor_tensor(
                diff96[:].rearrange("p (j c) -> p j c", c=3),
                pts96[:].rearrange("p (j c) -> p j c", c=3),
                selbb[:].unsqueeze(1).broadcast_to([128, NJ, 3]),
                op=ALU.subtract)
            nc.vector.tensor_mul(diffsq[:], diff96[:], diff96[:])
            nc.vector.tensor_reduce(
                d_newt[:], diffsq[:].rearrange("p (j c) -> p j c", c=3),
                axis=AX.X, op=ALU.add)

        for t in range(M):
            fps_select_update(t)

        nc.scalar.copy(centers[:], centers_ps[:])
        fps_es.close()

        if debug:
            nc.sync.dma_start(d_dbg_centers.ap(), centers[:])

        # ================= post-FPS per-core prep =================
        sc_es = ExitStack()
        sc_psum = sc_es.enter_context(
            tc.tile_pool(name="sc_psum", bufs=2, space="PSUM"))

        # centers_mine [16, 3]
        ps_cm = sc_psum.tile([MC, 3], F32, tag="cm")
        nc.tensor.matmul(ps_cm[:], sel16[:], centers[:], start=True, stop=True)
        nc.scalar.copy(cmine[:], ps_cm[:])
        # c2 and thresholds
        nc.vector.tensor_mul(cmsq[:], cmine[:], cmine[:])
        nc.vector.tensor_reduce(c2m[:], cmsq[:], axis=AX.X, op=ALU.add)
        # negthr = c2 - THR
        nc.vector.tensor_scalar(negthr[:], c2m[:], -THR, None, op0=ALU.add)
        # centersT_mine [3, 16]
        ps_ctm = sc_psum.tile([3, MC], F32, tag="ctm")
        nc.tensor.transpose(ps_ctm[:], cmine[:], ident[0:MC, 0:MC])
        nc.scalar.copy(ctm[:], ps_ctm[:])
        nc.scalar.mul(ctm2[:], ps_ctm[:], -2.0)
        # U2b [128, 16] = W1a^T @ centersT_mine + b1
        ps_u = sc_psum.tile([C, MC], F32, tag="u2b")
        nc.tensor.matmul(ps_u[:], W1a[:], ctm[:], start=True, stop=False)
        nc.tensor.matmul(ps_u[:], b1r[:], ones_1x16[:], start=False, stop=True)
        nc.scalar.copy(U2b[:], ps_u[:])
        # mask16 [16, N]: relu(q - 2 c.p + c2 - THR)  (0 iff valid)
        for ci in range(NCHUNK):
            sl = slice(ci * CHUNK, (ci + 1) * CHUNK)
            ps_m = sc_psum.tile([MC, CHUNK], F32, tag="m16")
            nc.tensor.matmul(ps_m[:], ones_3x16[:], pTsq[:, sl], start=True,
                             stop=False)
            nc.tensor.matmul(ps_m[:], ctm2[:], pT[:, sl], start=False,
                             stop=True)
            nc.scalar.activation(mask16[:, sl], ps_m[:], ACTF.Relu,
                                 bias=negthr[:], scale=1.0)
        sc_es.close()

        if debug:
            nc.sync.dma_start(d_dbg_mask.ap(), mask16[:])

        # ================= per-cluster MLP + masked max =================
        mlp_es = ExitStack()
        mlp_psum = mlp_es.enter_context(
            tc.tile_pool(name="mlp_psum", bufs=2, space="PSUM"))
        h1_pool = mlp_es.enter_context(tc.tile_pool(name="h1", bufs=10))
        gp_pool = mlp_es.enter_context(tc.tile_pool(name="gp", bufs=2))
        for j in range(MC):
            gparts = gp_pool.tile([C, 2], F32, tag="gparts")
            for grp in range(2):
                ps2 = mlp_psum.tile([C, GRP * CHUNK], F32, tag="ps2")
                h1list = []
                for q in range(GRP):
                    ci = grp * GRP + q
                    sl = slice(ci * CHUNK, (ci + 1) * CHUNK)
                    h1 = h1_pool.tile([C, CHUNK], F32R, tag="h1")
                    if q % 4 == 3:
                        nc.vector.tensor_scalar(h1[:], P3[:, sl],
                                                U2b[:, j:j + 1], 0.0,
                                                op0=ALU.add, op1=ALU.max)
                    else:
                        nc.scalar.activation(h1[:], P3[:, sl], ACTF.Relu,
                                             bias=U2b[:, j:j + 1], scale=1.0)
                    h1list.append(h1)
                for q in range(GRP):
                    qsl = slice(q * CHUNK, (q + 1) * CHUNK)
                    nc.tensor.matmul(ps2[:, qsl], W2r[:], h1list[q][:],
                                     start=True, stop=False)
                for q in range(GRP):
                    ci = grp * GRP + q
                    sl = slice(ci * CHUNK, (ci + 1) * CHUNK)
                    qsl = slice(q * CHUNK, (q + 1) * CHUNK)
                    nc.tensor.matmul(ps2[:, qsl],
                                     NB16[:, j * 128:(j + 1) * 128],
                                     mask16[:, sl], start=False,
                                     stop=True)
                nc.vector.tensor_reduce(gparts[:, grp:grp + 1], ps2[:],
                                        axis=AX.X, op=ALU.max)
            nc.vector.tensor_reduce(G[:, j:j + 1], gparts[:], axis=AX.X,
                                    op=ALU.max)
        mlp_es.close()

        if debug:
            nc.sync.dma_start(d_dbg_G.ap(), G[:])

        # ================= box MLP =================
        bx_es = ExitStack()
        bx_psum = bx_es.enter_context(
            tc.tile_pool(name="bx_psum", bufs=1, space="PSUM"))
        bx_pool = bx_es.enter_context(tc.tile_pool(name="bx", bufs=2))
        # g = relu(gmax + b2)
        nc.scalar.activation(Grelu[:], G[:], ACTF.Relu, bias=b2c[:], scale=1.0)
        ps_g3 = bx_psum.tile([C, MC], F32, tag="g3")
        nc.tensor.matmul(ps_g3[:], W3r[:], Grelu[:], start=True, stop=True)
        g3 = bx_pool.tile([C, MC], F32R, tag="g3s")
        nc.scalar.activation(g3[:], ps_g3[:], ACTF.Relu, bias=b3c[:],
                             scale=1.0)
        ps_g4 = bx_psum.tile([C, MC], F32, tag="g4")
        nc.tensor.matmul(ps_g4[:], W4r[:], g3[:], start=True, stop=True)
        g4 = bx_pool.tile([C, MC], F32R, tag="g4s")
        nc.scalar.activation(g4[:], ps_g4[:], ACTF.Relu, bias=b4c[:],
                             scale=1.0)
        ps_bx = bx_psum.tile([7, MC], F32, tag="bx")
        nc.tensor.matmul(ps_bx[:], Wfr[:], g4[:], start=True, stop=False)
        nc.tensor.matmul(ps_bx[:], bfr[:], ones_1x16[:], start=False,
                         stop=True)
        nc.scalar.copy(boxesT[:], ps_bx[:])
        bx_es.close()

        if debug:
            nc.sync.dma_start(d_dbg_boxesT.ap(), boxesT[:])

        # ================= AllGather box logits =================
        dram = es.enter_context(tc.tile_pool(name="dram", bufs=1, space="DRAM"))
        bounce_in = dram.tile([7, MC], F32)
        bounce_out = dram.tile([NCORES, 7 * MC], F32)
        nc.sync.dma_start(bounce_in[:], boxesT[:])
        nc.gpsimd.collective_compute(
            "AllGather", mybir.AluOpType.bypass,
            replica_groups=[list(range(NCORES))],
            ins=[bounce_in[:].opt()],
            outs=[bounce_out[:].opt()],
        )
        # reassemble: cluster m = 8j+k -> bounce_out[k, c*16+j]
        BTall = cp.tile([7, 128], F32)
        nc.sync.dma_start(
            BTall[:].rearrange("c (j k) -> c j k", k=NCORES),
            bounce_out[:].rearrange("k (c j) -> c j k", j=MC),
        )

        # ================= NMS =================
        nms_es = ExitStack()
        nms_psum = nms_es.enter_context(
            tc.tile_pool(name="nms_psum", bufs=1, space="PSUM"))
        # S7 = sigmoid(logits); BX = [sig | logits] transposed
        nc.scalar.activation(S14[0:7, :], BTall[:], ACTF.Sigmoid)
        ps_bxall = nms_psum.tile([128, 14], F32, tag="bxall")
        nc.tensor.transpose(ps_bxall[:, 0:7], S14[0:7, :], ident[0:7, 0:7])
        nc.tensor.transpose(ps_bxall[:, 7:14], BTall[:], ident[0:7, 0:7])
        nc.vector.tensor_copy(BX[:], ps_bxall[:])
        # cols of BX: 0 score-sig, 1..3 center, 4..6 dims, 7 score-logit
        # lo = c - 0.5 d ; hi = c + 0.5 d
        nc.vector.scalar_tensor_tensor(lo3[:], BX[:, 4:7], -0.5, BX[:, 1:4],
                                       op0=ALU.mult, op1=ALU.add)
        nc.vector.scalar_tensor_tensor(hi3[:], BX[:, 4:7], 0.5, BX[:, 1:4],
                                       op0=ALU.mult, op1=ALU.add)
        nc.vector.tensor_mul(vol[:], BX[:, 4:5], BX[:, 5:6])
        nc.vector.tensor_mul(vol[:], vol[:], BX[:, 6:7])
        # PR = [lo3 | hi3 | vol | score-logit]
        nc.vector.tensor_copy(PR[:, 0:3], lo3[:])
        nc.vector.tensor_copy(PR[:, 3:6], hi3[:])
        nc.vector.tensor_copy(PR[:, 6:7], vol[:])
        nc.vector.tensor_copy(PR[:, 7:8], BX[:, 7:8])
        ps_tp = nms_psum.tile([8, 128], F32, tag="tp")
        nc.tensor.transpose(ps_tp[:], PR[:], ident[:])
        nc.vector.tensor_copy(TPs[:], ps_tp[:])
        # broadcast all 8 rows: psumB[:, r*128:(r+1)*128] = row r over parts
        psB = nms_psum.tile([128, 8 * 128], F32, tag="psB")
        for r in range(8):
            nc.tensor.matmul(psB[:, r * 128:(r + 1) * 128],
                             ER[:, r * 128:(r + 1) * 128],
                             TPs[:], start=True, stop=True)

        def colB(r):
            return psB[:, r * 128:(r + 1) * 128]

        wrk = nms_es.enter_context(tc.tile_pool(name="nms_wrk", bufs=1))
        inter = wrk.tile([128, 128], F32, tag="inter")
        tmpA = wrk.tile([128, 128], F32, tag="tmpA")
        tmpB = wrk.tile([128, 128], F32, tag="tmpB")
        for c in range(3):
            # min(hi_i, hi_j)
            nc.vector.tensor_scalar(tmpA[:], colB(3 + c), hi3[:, c:c + 1],
                                    None, op0=ALU.min)
            # max(lo_i, lo_j)
            nc.vector.tensor_scalar(tmpB[:], colB(c), lo3[:, c:c + 1], None,
                                    op0=ALU.max)
            # w = relu(minhi - maxlo)
            nc.vector.scalar_tensor_tensor(tmpA[:], tmpB[:], -1.0, tmpA[:],
                                           op0=ALU.mult, op1=ALU.add)
            nc.vector.tensor_scalar_max(tmpA[:], tmpA[:], 0.0)
            if c == 0:
                nc.vector.tensor_copy(inter[:], tmpA[:])
            else:
                nc.vector.tensor_mul(inter[:], inter[:], tmpA[:])
        # volsum = vol_i + vol_j + 1e-8
        nc.vector.tensor_scalar(tmpB[:], colB(6), vol[:], 1e-8, op0=ALU.add,
                                op1=ALU.add)
        # D = volsum - inter
        nc.vector.scalar_tensor_tensor(tmpB[:], inter[:], -1.0, tmpB[:],
                                       op0=ALU.mult, op1=ALU.add)
        # P_iou = (4*inter > D)
        nc.vector.scalar_tensor_tensor(tmpA[:], inter[:], 1.0 / NMS_THR,
                                       tmpB[:], op0=ALU.mult, op1=ALU.is_gt)
        # P_score[i,j] = score_j < score_i
        nc.vector.tensor_scalar(tmpB[:], colB(7), BX[:, 7:8], None,
                                op0=ALU.is_lt)
        nc.vector.tensor_mul(P_s[:], tmpA[:], tmpB[:])
        # Jacobi fixpoint: keep_j = !any_i P[i,j] keep_i
        nc.vector.memset(keep[:], 1.0)
        ps_k = nms_psum.tile([128, 1], F32, tag="kps")
        for it in range(NMS_ITERS):
            nc.tensor.matmul(ps_k[:], P_s[:], keep[:], start=True, stop=True)
            nc.vector.tensor_scalar(keep[:], ps_k[:], 0.5, None, op0=ALU.is_lt)
        if debug:
            nc.sync.dma_start(d_dbg_keep.ap(), keep[:])
        # out = coords * keep
        nc.vector.tensor_scalar(outt[:], BX[:, 1:7], keep[:], None,
                                op0=ALU.mult)
        nc.sync.dma_start(d_out.ap(), outt[:])

        nms_es.close()
        es.close()

    nc.compile()
    return nc


def _prep_inputs(vote_points, vote_features, W1, b1, W2, b2, W3, b3, W4, b4,
                 Wf, bf):
    """Pure layout transforms of the full inputs -> per-core input maps."""
    f32 = np.float32
    pts = np.ascontiguousarray(vote_points, dtype=f32)
    feat = np.ascontiguousarray(vote_features, dtype=f32)
    base = {
        "pts96": pts.reshape(128, 96).copy(),
        "pT": pts.T.copy(),
        "featT": feat.T.copy(),
        "W1a": np.ascontiguousarray(W1[:3], f32),
        "W1b": np.ascontiguousarray(W1[3:], f32),
        "W2": np.ascontiguousarray(W2, f32),
        "W3": np.ascontiguousarray(W3, f32),
        "W4": np.ascontiguousarray(W4, f32),
        "Wf": np.ascontiguousarray(Wf, f32),
        "b1r": np.ascontiguousarray(b1, f32).reshape(1, C),
        "b2c": np.ascontiguousarray(b2, f32).reshape(C, 1),
        "b3c": np.ascontiguousarray(b3, f32).reshape(C, 1),
        "b4c": np.ascontiguousarray(b4, f32).reshape(C, 1),
        "bfr": np.ascontiguousarray(bf, f32).reshape(1, 7),
    }
    in_maps = []
    for k in range(NCORES):
        m = dict(base)
        sel = np.zeros((128, MC), f32)
        for j in range(MC):
            sel[NCORES * j + k, j] = 1.0
        m["sel16"] = sel
        in_maps.append(m)
    return in_maps


def kernel(**inputs):
    from concourse.bass_utils import run_bass_kernel_spmd

    if "nc" not in _cache:
        _cache["nc"] = _build(debug=False)
    nc = _cache["nc"]
    in_maps = _prep_inputs(**inputs)
    res = run_bass_kernel_spmd(nc, in_maps, core_ids=list(range(NCORES)))
    out = np.asarray(res.results[0]["out"], dtype=np.float32)
    return out


# revision 20
# speedup vs baseline: 1.5193x; 1.0394x over previous
"""Trainium2 Bass kernel for nn_DetectionHead (VoteNet-style detection head).

Self-contained: builds an 8-core SPMD Bass/Tile kernel, shards the M=128
clusters across cores (interleaved mod 8), replicates FPS + NMS, and
AllGathers the per-core box logits for the final NMS pass.

kernel(**inputs) takes the full unsharded inputs and returns the full
[128, 6] output.
"""

import numpy as np

NCORES = 8
N = 4096          # points
C = 128           # feature channels
M = 128           # clusters
MC = M // NCORES  # clusters per core (16)
NJ = 32           # FPS free-dim (N = 128 * NJ)
RADIUS = 0.5
THR = RADIUS * RADIUS   # 0.25 (d2 < THR)
NMS_THR = 0.25
BIG = 1.0e7
NMS_ITERS = 6
CHUNK = 512
NCHUNK = N // CHUNK       # 8
GRP = 4                   # psum2 groups of 4 chunks -> [128, 2048]

_cache = {}


def _build(debug=False):
    import concourse.bacc as bacc
    import concourse.tile as tile
    import concourse.mybir as mybir
    import concourse.bass_isa as bass_isa

    F32 = mybir.dt.float32
    F32R = mybir.dt.float32r
    BF16 = mybir.dt.bfloat16
    I32 = mybir.dt.int32
    ALU = mybir.AluOpType
    ACTF = mybir.ActivationFunctionType
    AX = mybir.AxisListType

    nc = bacc.Bacc("TRN2", target_bir_lowering=False, debug=False,
                   num_devices=NCORES)

    # ---- DRAM I/O ----
    d_pts96 = nc.dram_tensor("pts96", [128, 96], F32, kind="ExternalInput")
    d_pT = nc.dram_tensor("pT", [3, N], F32, kind="ExternalInput")
    d_featT = nc.dram_tensor("featT", [C, N], F32, kind="ExternalInput")
    d_W1a = nc.dram_tensor("W1a", [3, C], F32, kind="ExternalInput")
    d_W1b = nc.dram_tensor("W1b", [C, C], F32, kind="ExternalInput")
    d_W2 = nc.dram_tensor("W2", [C, C], F32, kind="ExternalInput")
    d_W3 = nc.dram_tensor("W3", [C, C], F32, kind="ExternalInput")
    d_W4 = nc.dram_tensor("W4", [C, C], F32, kind="ExternalInput")
    d_Wf = nc.dram_tensor("Wf", [C, 7], F32, kind="ExternalInput")
    d_b1r = nc.dram_tensor("b1r", [1, C], F32, kind="ExternalInput")
    d_b2c = nc.dram_tensor("b2c", [C, 1], F32, kind="ExternalInput")
    d_b3c = nc.dram_tensor("b3c", [C, 1], F32, kind="ExternalInput")
    d_b4c = nc.dram_tensor("b4c", [C, 1], F32, kind="ExternalInput")
    d_bfr = nc.dram_tensor("bfr", [1, 7], F32, kind="ExternalInput")
    d_sel16 = nc.dram_tensor("sel16", [128, MC], F32, kind="ExternalInput")

    d_out = nc.dram_tensor("out", [M, 6], F32, kind="ExternalOutput")
    if debug:
        d_dbg_centers = nc.dram_tensor("dbg_centers", [M, 3], F32,
                                       kind="ExternalOutput")
        d_dbg_G = nc.dram_tensor("dbg_G", [C, MC], F32, kind="ExternalOutput")
        d_dbg_boxesT = nc.dram_tensor("dbg_boxesT", [7, MC], F32,
                                      kind="ExternalOutput")
        d_dbg_mask = nc.dram_tensor("dbg_mask", [MC, N], F32,
                                    kind="ExternalOutput")
        d_dbg_keep = nc.dram_tensor("dbg_keep", [M, 1], F32,
                                    kind="ExternalOutput")

    from contextlib import ExitStack
    es = ExitStack()
    with tile.TileContext(nc) as tc:
        cp = es.enter_context(tc.tile_pool(name="const", bufs=1))
        stage_es = ExitStack()
        stage_pool = stage_es.enter_context(tc.tile_pool(name="stage", bufs=1))
        featT_stage = stage_pool.tile([C, N], F32)
        # ---- constant / persistent tiles ----
        pts96 = cp.tile([128, 96], F32)
        pT = cp.tile([3, N], F32)
        pTsq = cp.tile([3, N], F32)
        featT = cp.tile([C, N], F32R)
        pTr = cp.tile([3, N], F32R)
        P3 = cp.tile([C, N], F32R)
        mask16 = cp.tile([MC, N], BF16)
        W1a = cp.tile([3, C], F32)
        W1am2 = cp.tile([3, C], F32R)
        W1br = cp.tile([C, C], F32R)
        W2r = cp.tile([C, C], F32R)
        W3r = cp.tile([C, C], F32R)
        W4r = cp.tile([C, C], F32R)
        Wfr = cp.tile([C, 7], F32R)
        W1b = cp.tile([C, C], F32)
        W2 = cp.tile([C, C], F32)
        W3 = cp.tile([C, C], F32)
        W4 = cp.tile([C, C], F32)
        Wf = cp.tile([C, 7], F32)
        b1r = cp.tile([1, C], F32)
        b2c = cp.tile([C, 1], F32)
        b3c = cp.tile([C, 1], F32)
        b4c = cp.tile([C, 1], F32)
        bfr = cp.tile([1, 7], F32)
        sel16 = cp.tile([128, MC], F32)
        ident = cp.tile([128, 128], F32)
        ident_i = cp.tile([128, 128], I32)
        ones_1x128 = cp.tile([1, 128], F32)
        ones_1x16 = cp.tile([1, MC], F32)
        ones_3x16 = cp.tile([3, MC], F32)
        negbig = cp.tile([1, C], F32)
        NB8 = cp.tile([8, 8 * 128], BF16)
        NB8_i = cp.tile([8, 8 * 128], I32)
        ER = cp.tile([8, 8 * 128], F32)
        ER_i = cp.tile([8, 8 * 128], I32)
        IDrow = cp.tile([1, 256], F32)
        IDrow_i = cp.tile([1, 256], I32)
        centersA = cp.tile([128, 3], F32)
        centersB = cp.tile([128, 3], F32)
        U2bh = [cp.tile([C, 8], F32, name=f"U2b{h}") for h in range(2)]
        ctmh = [cp.tile([3, 8], F32, name=f"ctm{h}") for h in range(2)]
        ctm2h = [cp.tile([3, 8], F32, name=f"ctm2{h}") for h in range(2)]
        cmineh = [cp.tile([8, 3], F32, name=f"cmine{h}") for h in range(2)]
        cmsqh = [cp.tile([8, 3], F32, name=f"cmsq{h}") for h in range(2)]
        c2mh = [cp.tile([8, 1], F32, name=f"c2m{h}") for h in range(2)]
        negthrh = [cp.tile([8, 1], F32, name=f"negthr{h}") for h in range(2)]
        mask16h = [cp.tile([8, N], BF16, name=f"mask16{h}") for h in range(2)]
        G = cp.tile([C, MC], F32)
        Greluh = [cp.tile([C, 8], F32R, name=f"Grelu{h}") for h in range(2)]
        boxesTh = [cp.tile([7, 8], F32, name=f"boxesT{h}") for h in range(2)]
        # FPS state
        min_d = cp.tile([128, NJ], F32)
        d_newt = cp.tile([128, NJ], F32)
        rowmax = cp.tile([128, 1], F32)
        gb = cp.tile([128, 1], F32)
        partials = cp.tile([128, 3], F32)
        selbb = cp.tile([128, 3], F32)
        masked96 = cp.tile([128, 96], F32)
        diff96 = cp.tile([128, 96], F32)
        diffsq = cp.tile([128, 96], F32)
        iotaN_i = cp.tile([128, NJ], I32)
        # NMS tiles
        S14 = cp.tile([14, 128], F32)
        BX = cp.tile([128, 14], F32)
        PR = cp.tile([128, 8], F32)
        TPs = cp.tile([8, 128], F32)
        P_s = cp.tile([128, 128], F32)
        keep = cp.tile([128, 1], F32)
        lo3 = cp.tile([128, 3], F32)
        hi3 = cp.tile([128, 3], F32)
        vol = cp.tile([128, 1], F32)
        outt = cp.tile([128, 6], F32)

        # ---- input DMA ----
        nc.sync.dma_start(pts96[:], d_pts96.ap())
        nc.sync.dma_start(pT[:], d_pT.ap())
        nc.sync.dma_start(featT_stage[:], d_featT.ap())
        nc.sync.dma_start(W1a[:], d_W1a.ap())
        nc.sync.dma_start(W1b[:], d_W1b.ap())
        nc.sync.dma_start(W2[:], d_W2.ap())
        nc.sync.dma_start(W3[:], d_W3.ap())
        nc.sync.dma_start(W4[:], d_W4.ap())
        nc.sync.dma_start(Wf[:], d_Wf.ap())
        nc.sync.dma_start(b1r[:], d_b1r.ap())
        nc.sync.dma_start(b2c[:], d_b2c.ap())
        nc.sync.dma_start(b3c[:], d_b3c.ap())
        nc.sync.dma_start(b4c[:], d_b4c.ap())
        nc.sync.dma_start(bfr[:], d_bfr.ap())
        nc.sync.dma_start(sel16[:], d_sel16.ap())

        # ---- constants ----
        nc.gpsimd.iota(ident_i[:], pattern=[[1, 128]], base=0,
                       channel_multiplier=-1)
        nc.vector.tensor_scalar(ident[:], ident_i[:], 0, None,
                                op0=ALU.is_equal)
        nc.vector.memset(ones_1x128[:], 1.0)
        nc.vector.memset(ones_1x16[:], 1.0)
        nc.vector.memset(ones_3x16[:], 1.0)
        nc.vector.memset(negbig[:], -BIG)
        nc.gpsimd.iota(NB8_i[:].rearrange("p (j c) -> p j c", c=128),
                       pattern=[[1, 8], [0, 128]], base=0,
                       channel_multiplier=-1)
        nc.vector.tensor_scalar(NB8[:], NB8_i[:], 0, -BIG,
                                op0=ALU.is_equal, op1=ALU.mult)
        nc.gpsimd.iota(ER_i[:].rearrange("p (j c) -> p j c", c=128),
                       pattern=[[1, 8], [0, 128]], base=0,
                       channel_multiplier=-1)
        nc.vector.tensor_scalar(ER[:], ER_i[:], 0, None, op0=ALU.is_equal)
        nc.gpsimd.iota(IDrow_i[:], pattern=[[1, 256]], base=0,
                       channel_multiplier=0)
        nc.vector.tensor_scalar(IDrow[:], IDrow_i[:], 127, None,
                                op0=ALU.is_equal)
        nc.vector.tensor_scalar_mul(W1am2[:], W1a[:], -2.0)
        nc.vector.tensor_copy(featT[:], featT_stage[:])
        nc.vector.tensor_copy(W1br[:], W1b[:])
        nc.vector.tensor_copy(W2r[:], W2[:])
        nc.vector.tensor_copy(W3r[:], W3[:])
        nc.vector.tensor_copy(W4r[:], W4[:])
        nc.vector.tensor_copy(Wfr[:], Wf[:])
        nc.scalar.copy(pTr[:], pT[:])
        nc.gpsimd.iota(iotaN_i[:], pattern=[[1, NJ]], base=0,
                       channel_multiplier=NJ)
        nc.vector.tensor_scalar(min_d[:], iotaN_i[:], -1.0, None,
                                op0=ALU.mult)
        nc.vector.tensor_mul(pTsq[:], pT[:], pT[:])

        # views of pts96
        pts_pjc = pts96[:].rearrange("p (j c) -> p j c", c=3)

        stage_es.close()
        # ================= FPS =================
        fps_es = ExitStack()
        p3_psum = fps_es.enter_context(
            tc.tile_pool(name="p3_psum", bufs=1, space="PSUM"))
        ctr_psum = fps_es.enter_context(
            tc.tile_pool(name="ctr_psum", bufs=1, space="PSUM"))
        srow_pool = fps_es.enter_context(tc.tile_pool(name="srow", bufs=3))
        ctr_psA = ctr_psum.tile([128, 3], F32, tag="ctrA")
        ctr_psB = ctr_psum.tile([128, 3], F32, tag="ctrB")

        # ---- P3 = W1b^T @ featT + (-2 W1a)^T @ pT  (cluster independent) ----
        for ci in range(NCHUNK):
            sl = slice(ci * CHUNK, (ci + 1) * CHUNK)
            ps = p3_psum.tile([C, CHUNK], F32, tag="p3ps")
            nc.tensor.matmul(ps[:], W1br[:], featT[:, sl], start=True,
                             stop=False)
            nc.tensor.matmul(ps[:], W1am2[:], pTr[:, sl], start=False,
                             stop=True)
            nc.scalar.copy(P3[:, sl], ps[:])

        def fps_select_update(t):
            """Select center t from min_d (+pending d_newt), update state."""
            if t == 0:
                pass  # min_d holds -n (selects point 0)
            elif t == 1:
                nc.vector.tensor_copy(min_d[:], d_newt[:])
            else:
                nc.vector.tensor_tensor(min_d[:], min_d[:], d_newt[:],
                                        op=ALU.min)
            nc.vector.tensor_reduce(rowmax[:], min_d[:], axis=AX.X,
                                    op=ALU.max)
            nc.gpsimd.partition_all_reduce(gb[:], rowmax[:], channels=128,
                                           reduce_op=bass_isa.ReduceOp.max)
            # masked96 = (min_d >= gmax) * pts   (global one-hot mask)
            nc.vector.scalar_tensor_tensor(
                out=masked96[:].rearrange("p (j c) -> p c j", c=3),
                in0=min_d[:].unsqueeze(1).broadcast_to([128, 3, NJ]),
                scalar=gb[:],
                in1=pts96[:].rearrange("p (j c) -> p c j", c=3),
                op0=ALU.is_ge, op1=ALU.mult)
            nc.vector.tensor_reduce(
                partials[:], masked96[:].rearrange("p (j c) -> p c j", c=3),
                axis=AX.X, op=ALU.add)
            nc.gpsimd.partition_all_reduce(selbb[:], partials[:], channels=128,
                                           reduce_op=bass_isa.ReduceOp.add)
            # record center: ctr_ps(A|B) += e_t (x) c_t
            srow = srow_pool.tile([1, 3], F32, tag="srow")
            nc.scalar.copy(srow[:], selbb[0:1, :])
            ctr = ctr_psA if t < 64 else ctr_psB
            nc.tensor.matmul(ctr[:],
                             IDrow[0:1, 127 - t:255 - t], srow[:],
                             start=(t in (0, 64)), stop=(t in (63, M - 1)),
                             skip_group_check=True)
            if t == M - 1:
                return
            # d_newt = sum_c (pts - c_t)^2
            nc.vector.tensor_tensor(
                diff96[:].rearrange("p (j c) -> p j c", c=3),
                pts96[:].rearrange("p (j c) -> p j c", c=3),
                selbb[:].unsqueeze(1).broadcast_to([128, NJ, 3]),
                op=ALU.subtract)
            nc.vector.tensor_mul(diffsq[:], diff96[:], diff96[:])
            nc.vector.tensor_reduce(
                d_newt[:], diffsq[:].rearrange("p (j c) -> p j c", c=3),
                axis=AX.X, op=ALU.add)

        for t in range(M):
            fps_select_update(t)

        nc.scalar.copy(centersA[:], ctr_psA[:])
        nc.scalar.copy(centersB[:], ctr_psB[:])

        # shared pools for both halves (PSUM budget: ctr2+p3$1+sc2+mlp2+bx1=8)
        sc_es = ExitStack()
        sc_psum = sc_es.enter_context(
            tc.tile_pool(name="sc_psum", bufs=2, space="PSUM"))
        mlp_es = ExitStack()
        mlp_psum = mlp_es.enter_context(
            tc.tile_pool(name="mlp_psum", bufs=2, space="PSUM"))
        h1_pool = mlp_es.enter_context(tc.tile_pool(name="h1", bufs=10))
        gp_pool = mlp_es.enter_context(tc.tile_pool(name="gp", bufs=2))
        bx_es = ExitStack()
        bx_psum = bx_es.enter_context(
            tc.tile_pool(name="bx_psum", bufs=1, space="PSUM"))
        bx_pool = bx_es.enter_context(tc.tile_pool(name="bx", bufs=2))
        dram = es.enter_context(tc.tile_pool(name="dram", bufs=1,
                                             space="DRAM"))
        bounce_outs = []

        def stage_c(h, centersX):
            """Per-half cluster prep: thresholds, U2b bias cols, mask rows."""
            ps_cm = sc_psum.tile([8, 3], F32, tag="sc")
            nc.tensor.matmul(ps_cm[:], sel16[:, 8 * h:8 * h + 8], centersX[:],
                             start=True, stop=True)
            nc.scalar.copy(cmineh[h][:], ps_cm[:])
            nc.vector.tensor_mul(cmsqh[h][:], cmineh[h][:], cmineh[h][:])
            nc.vector.tensor_reduce(c2mh[h][:], cmsqh[h][:], axis=AX.X,
                                    op=ALU.add)
            nc.vector.tensor_scalar(negthrh[h][:], c2mh[h][:], -THR, None,
                                    op0=ALU.add)
            ps_ctm = sc_psum.tile([3, 8], F32, tag="sc")
            nc.tensor.transpose(ps_ctm[:], cmineh[h][:], ident[0:8, 0:8])
            nc.scalar.copy(ctmh[h][:], ps_ctm[:])
            nc.scalar.mul(ctm2h[h][:], ps_ctm[:], -2.0)
            ps_u = sc_psum.tile([C, 8], F32, tag="sc")
            nc.tensor.matmul(ps_u[:], W1a[:], ctmh[h][:], start=True,
                             stop=False)
            nc.tensor.matmul(ps_u[:], b1r[:], ones_1x16[:, 0:8], start=False,
                             stop=True)
            nc.scalar.copy(U2bh[h][:], ps_u[:])
            for ci in range(NCHUNK):
                sl = slice(ci * CHUNK, (ci + 1) * CHUNK)
                ps_m = sc_psum.tile([8, CHUNK], F32, tag="sc")
                nc.tensor.matmul(ps_m[:], ones_3x16[:, 0:8], pTsq[:, sl],
                                 start=True, stop=False)
                nc.tensor.matmul(ps_m[:], ctm2h[h][:], pT[:, sl], start=False,
                                 stop=True)
                nc.scalar.activation(mask16h[h][:, sl], ps_m[:], ACTF.Relu,
                                     bias=negthrh[h][:], scale=1.0)

        def mlp_half(h):
            for jl in range(8):
                j = 8 * h + jl
                gparts = gp_pool.tile([C, NCHUNK], F32, tag="gparts")
                for ci in range(NCHUNK):
                    sl = slice(ci * CHUNK, (ci + 1) * CHUNK)
                    h1 = h1_pool.tile([C, CHUNK], F32R, tag="h1")
                    nc.scalar.activation(h1[:], P3[:, sl], ACTF.Relu,
                                         bias=U2bh[h][:, jl:jl + 1],
                                         scale=1.0)
                    ps2 = mlp_psum.tile([C, CHUNK], F32, tag="ps2")
                    nc.tensor.matmul(ps2[:], W2r[:], h1[:], start=True,
                                     stop=False)
                    nc.tensor.matmul(ps2[:], NB8[:, jl * 128:(jl + 1) * 128],
                                     mask16h[h][:, sl], start=False,
                                     stop=True)
                    nc.vector.tensor_reduce(gparts[:, ci:ci + 1], ps2[:],
                                            axis=AX.X, op=ALU.max)
                nc.vector.tensor_reduce(G[:, j:j + 1], gparts[:], axis=AX.X,
                                        op=ALU.max)

        def boxes_half(h):
            nc.scalar.activation(Greluh[h][:], G[:, 8 * h:8 * h + 8],
                                 ACTF.Relu, bias=b2c[:], scale=1.0)
            ps_g3 = bx_psum.tile([C, 8], F32, tag="bx")
            nc.tensor.matmul(ps_g3[:], W3r[:], Greluh[h][:], start=True,
                             stop=True)
            g3 = bx_pool.tile([C, 8], F32R, tag="g3s")
            nc.scalar.activation(g3[:], ps_g3[:], ACTF.Relu, bias=b3c[:],
                                 scale=1.0)
            ps_g4 = bx_psum.tile([C, 8], F32, tag="bx")
            nc.tensor.matmul(ps_g4[:], W4r[:], g3[:], start=True, stop=True)
            g4 = bx_pool.tile([C, 8], F32R, tag="g4s")
            nc.scalar.activation(g4[:], ps_g4[:], ACTF.Relu, bias=b4c[:],
                                 scale=1.0)
            ps_bx = bx_psum.tile([7, 8], F32, tag="bx")
            nc.tensor.matmul(ps_bx[:], Wfr[:], g4[:], start=True, stop=False)
            nc.tensor.matmul(ps_bx[:], bfr[:], ones_1x16[:, 0:8], start=False,
                             stop=True)
            nc.scalar.copy(boxesTh[h][:], ps_bx[:])
            bounce_in = dram.tile([7, 8], F32, name=f"bnc_in{h}")
            bounce_out = dram.tile([NCORES, 7 * 8], F32, name=f"bnc_out{h}")
            nc.sync.dma_start(bounce_in[:], boxesTh[h][:])
            nc.gpsimd.collective_compute(
                "AllGather", mybir.AluOpType.bypass,
                replica_groups=[list(range(NCORES))],
                ins=[bounce_in[:].opt()],
                outs=[bounce_out[:].opt()],
            )
            bounce_outs.append(bounce_out)

        stage_c(0, centersA)
        mlp_half(0)
        boxes_half(0)
        stage_c(1, centersB)
        mlp_half(1)
        boxes_half(1)
        bx_es.close()
        mlp_es.close()
        sc_es.close()
        fps_es.close()

        # reassemble: half h covers clusters m = 64h..64h+63; within a half,
        # cluster m = 64h + 8*jl + k lives at bounce_out[k, c*8+jl]
        BTall = cp.tile([7, 128], F32)
        for h in range(2):
            nc.sync.dma_start(
                BTall[:, 64 * h:64 * h + 64].rearrange(
                    "c (j k) -> c j k", k=NCORES),
                bounce_outs[h][:].rearrange("k (c j) -> c j k", j=8),
            )

        # ================= NMS =================
        nms_es = ExitStack()
        nms_psum = nms_es.enter_context(
            tc.tile_pool(name="nms_psum", bufs=1, space="PSUM"))
        # S7 = sigmoid(logits); BX = [sig | logits] transposed
        nc.scalar.activation(S14[0:7, :], BTall[:], ACTF.Sigmoid)
        ps_bxall = nms_psum.tile([128, 14], F32, tag="bxall")
        nc.tensor.transpose(ps_bxall[:, 0:7], S14[0:7, :], ident[0:7, 0:7])
        nc.tensor.transpose(ps_bxall[:, 7:14], BTall[:], ident[0:7, 0:7])
        nc.vector.tensor_copy(BX[:], ps_bxall[:])
        # cols of BX: 0 score-sig, 1..3 center, 4..6 dims, 7 score-logit
        # lo = c - 0.5 d ; hi = c + 0.5 d
        nc.vector.scalar_tensor_tensor(lo3[:], BX[:, 4:7], -0.5, BX[:, 1:4],
                                       op0=ALU.mult, op1=ALU.add)
        nc.vector.scalar_tensor_tensor(hi3[:], BX[:, 4:7], 0.5, BX[:, 1:4],
                                       op0=ALU.mult, op1=ALU.add)
        nc.vector.tensor_mul(vol[:], BX[:, 4:5], BX[:, 5:6])
        nc.vector.tensor_mul(vol[:], vol[:], BX[:, 6:7])
        # PR = [lo3 | hi3 | vol | score-logit]
        nc.vector.tensor_copy(PR[:, 0:3], lo3[:])
        nc.vector.tensor_copy(PR[:, 3:6], hi3[:])
        nc.vector.tensor_copy(PR[:, 6:7], vol[:])
        nc.vector.tensor_copy(PR[:, 7:8], BX[:, 7:8])
        ps_tp = nms_psum.tile([8, 128], F32, tag="tp")
        nc.tensor.transpose(ps_tp[:], PR[:], ident[:])
        nc.vector.tensor_copy(TPs[:], ps_tp[:])
        # broadcast all 8 rows: psumB[:, r*128:(r+1)*128] = row r over parts
        psB = nms_psum.tile([128, 8 * 128], F32, tag="psB")
        for r in range(8):
            nc.tensor.matmul(psB[:, r * 128:(r + 1) * 128],
                             ER[:, r * 128:(r + 1) * 128],
                             TPs[:], start=True, stop=True)

        def colB(r):
            return psB[:, r * 128:(r + 1) * 128]

        wrk = nms_es.enter_context(tc.tile_pool(name="nms_wrk", bufs=1))
        inter = wrk.tile([128, 128], F32, tag="inter")
        tmpA = wrk.tile([128, 128], F32, tag="tmpA")
        tmpB = wrk.tile([128, 128], F32, tag="tmpB")
        for c in range(3):
            # min(hi_i, hi_j)
            nc.vector.tensor_scalar(tmpA[:], colB(3 + c), hi3[:, c:c + 1],
                                    None, op0=ALU.min)
            # max(lo_i, lo_j)
            nc.vector.tensor_scalar(tmpB[:], colB(c), lo3[:, c:c + 1], None,
                                    op0=ALU.max)
            # w = relu(minhi - maxlo)
            nc.vector.scalar_tensor_tensor(tmpA[:], tmpB[:], -1.0, tmpA[:],
                                           op0=ALU.mult, op1=ALU.add)
            nc.vector.tensor_scalar_max(tmpA[:], tmpA[:], 0.0)
            if c == 0:
                nc.vector.tensor_copy(inter[:], tmpA[:])
            else:
                nc.vector.tensor_mul(inter[:], inter[:], tmpA[:])
        # volsum = vol_i + vol_j + 1e-8
        nc.vector.tensor_scalar(tmpB[:], colB(6), vol[:], 1e-8, op0=ALU.add,
                                op1=ALU.add)
        # D = volsum - inter
        nc.vector.scalar_tensor_tensor(tmpB[:], inter[:], -1.0, tmpB[:],
                                       op0=ALU.mult, op1=ALU.add)
        # P_iou = (4*inter > D)
        nc.vector.scalar_tensor_tensor(tmpA[:], inter[:], 1.0 / NMS_THR,
                                       tmpB[:], op0=ALU.mult, op1=ALU.is_gt)
        # P_score[i,j] = score_j < score_i
        nc.vector.tensor_scalar(tmpB[:], colB(7), BX[:, 7:8], None,
                                op0=ALU.is_lt)
        nc.vector.tensor_mul(P_s[:], tmpA[:], tmpB[:])
        # Jacobi fixpoint: keep_j = !any_i P[i,j] keep_i
        nc.vector.memset(keep[:], 1.0)
        ps_k = nms_psum.tile([128, 1], F32, tag="kps")
        for it in range(NMS_ITERS):
            nc.tensor.matmul(ps_k[:], P_s[:], keep[:], start=True, stop=True)
            nc.vector.tensor_scalar(keep[:], ps_k[:], 0.5, None, op0=ALU.is_lt)
        if debug:
            nc.sync.dma_start(d_dbg_keep.ap(), keep[:])
        # out = coords * keep
        nc.vector.tensor_scalar(outt[:], BX[:, 1:7], keep[:], None,
                                op0=ALU.mult)
        nc.sync.dma_start(d_out.ap(), outt[:])

        nms_es.close()
        es.close()

    nc.compile()
    return nc


def _prep_inputs(vote_points, vote_features, W1, b1, W2, b2, W3, b3, W4, b4,
                 Wf, bf):
    """Pure layout transforms of the full inputs -> per-core input maps."""
    f32 = np.float32
    pts = np.ascontiguousarray(vote_points, dtype=f32)
    feat = np.ascontiguousarray(vote_features, dtype=f32)
    base = {
        "pts96": pts.reshape(128, 96).copy(),
        "pT": pts.T.copy(),
        "featT": feat.T.copy(),
        "W1a": np.ascontiguousarray(W1[:3], f32),
        "W1b": np.ascontiguousarray(W1[3:], f32),
        "W2": np.ascontiguousarray(W2, f32),
        "W3": np.ascontiguousarray(W3, f32),
        "W4": np.ascontiguousarray(W4, f32),
        "Wf": np.ascontiguousarray(Wf, f32),
        "b1r": np.ascontiguousarray(b1, f32).reshape(1, C),
        "b2c": np.ascontiguousarray(b2, f32).reshape(C, 1),
        "b3c": np.ascontiguousarray(b3, f32).reshape(C, 1),
        "b4c": np.ascontiguousarray(b4, f32).reshape(C, 1),
        "bfr": np.ascontiguousarray(bf, f32).reshape(1, 7),
    }
    in_maps = []
    for k in range(NCORES):
        m = dict(base)
        sel = np.zeros((128, MC), f32)
        for j in range(MC):
            sel[NCORES * j + k, j] = 1.0
        m["sel16"] = sel
        in_maps.append(m)
    return in_maps


def kernel(**inputs):
    from concourse.bass_utils import run_bass_kernel_spmd

    if "nc" not in _cache:
        _cache["nc"] = _build(debug=False)
    nc = _cache["nc"]
    in_maps = _prep_inputs(**inputs)
    res = run_bass_kernel_spmd(nc, in_maps, core_ids=list(range(NCORES)))
    out = np.asarray(res.results[0]["out"], dtype=np.float32)
    return out
